# revision 23
# baseline (speedup 1.0000x reference)
"""Distributed causal attention block (QKV + RoPE + SDPA + Wo) on 8 TRN2 cores.

Tensor-parallel over heads (2 heads/core). Redesign vs baseline: no PE
transposes anywhere.

  phase 1: x^T tiles come from the DMA XBAR transpose engine; the QKV
           projection computes q^T/k^T directly ([head_dim, t] layout,
           lhsT = W^T tile) and v in [t, e] layout (lhsT = x^T tile).
           RoPE runs in the transposed layout: per head the 128 rows are
           [evens; odds] (host-permuted W columns), the e/o cross terms
           read the PSUM with swapped partition halves (mixed-space DVE
           ops), the sign of sin folded into the table: rot = q*C2 + sw*S2.
  phase 2: scores computed TRANSPOSED (s^T[k, q] = kt^T @ qt), exp on the
           scalar engine straight into bf16 P^T tiles (PV rhs), causal
           handled by a [128,128] transposed-triangle mask add on diagonal
           blocks plus column trimming. Softmax row sums via ones-column
           matmuls (M=1) accumulated in PSUM; normalization applied to the
           (tiny) attention output: o^T * broadcast(1/rowsum).
  phase 3: AllGather attention outputs (progressive pieces) -> Wo e-slice.
Host concatenates the 8 e-slices.
"""
import numpy as np
import ml_dtypes
import bass_rust
import concourse.bass as bass
import concourse.mybir as mybir
from concourse.tile import TileContext, add_dep_helper
from concourse.masks import make_identity

B, L, D, H = 2, 2048, 2048, 16
HD = 128
N_CORES = 8
HPC = H // N_CORES          # heads per core = 2
ES = HPC * HD               # 256 = e-slice width per core
T = B * L                   # 4096 tokens
P = 128
CH = 512                    # t-chunk
NCH = T // CH               # 8 chunks
N_DT = D // P               # 16 d-tiles
SCALE = 1.0 / float(np.sqrt(HD))
NEG = -30000.0              # causal fill; exp(SCALE*(s+NEG)) underflows to 0
FP = mybir.dt.float32
BF = mybir.dt.bfloat16

# attention-out AllGather pieces per batch, in units of 512-t q-chunks
AG_PIECES = {0: [(0, 2), (2, 4)], 1: [(0, 2), (2, 3), (3, 4)]}


def split_multi_waits(nc):
    """This walrus build allows 1 sync wait per instruction (2 for
    EventSemaphore). Tile attaches more on some instructions; hoist the
    extras onto same-engine NoOps."""
    for f in nc.m.functions:
        for bb in f.blocks:
            new_insts = []
            changed = False
            for ins in bb.instructions:
                si = ins.sync_info
                cap = 2 if type(ins).__name__ == "InstEventSemaphore" else 1
                if si is not None and len(si.on_wait) > cap:
                    waits = list(si.on_wait)
                    for k, w in enumerate(waits[cap:]):
                        new_insts.append(mybir.InstNoOp(
                            name=f"{ins.name}-wsplit{k}", ins=[], outs=[],
                            engine=ins.engine,
                            sync_info=bass_rust.SyncInfo(on_wait=[w], on_update=[]),
                        ))
                    ins.sync_info = bass_rust.SyncInfo(
                        on_wait=waits[:cap], on_update=list(si.on_update))
                    changed = True
                new_insts.append(ins)
            if changed:
                bb.instructions.clear()
                for i2 in new_insts:
                    bb.add_instruction(i2)


def make_tri_mask(nc, ap):
    """ap[k, q] = 0 where k <= q, NEG where k > q (transposed causal)."""
    nc.gpsimd.memset(ap, 0.0)
    nc.gpsimd.affine_select(
        out=ap, in_=ap,
        compare_op=mybir.AluOpType.is_ge,
        fill=NEG, base=0,
        # keep where (-1*x + 1*y) >= 0 i.e. q >= k
        pattern=[[1, ap.shape[1]]],
        channel_multiplier=-1,
    )


def build(debug=False):
    nc = bass.Bass()
    x_c = nc.declare_dram_parameter("x_c", [N_DT * CH, P], BF, isOutput=False)
    wqkvT = nc.declare_dram_parameter("wqkvT", [D, 3 * ES], BF, isOutput=False)
    c2_p = nc.declare_dram_parameter("c2_p", [HPC * P, L], BF, isOutput=False)
    s2_p = nc.declare_dram_parameter("s2_p", [HPC * P, L], BF, isOutput=False)
    woT = nc.declare_dram_parameter("woT", [D, ES], BF, isOutput=False)
    out = nc.declare_dram_parameter("out", [ES, T], FP, isOutput=True)
    if debug:
        dbg_qt = nc.declare_dram_parameter("dbg_qt", [P, HPC * T], FP,
                                           isOutput=True)
        dbg_kt = nc.declare_dram_parameter("dbg_kt", [P, HPC * T], FP,
                                           isOutput=True)
        dbg_v = nc.declare_dram_parameter("dbg_v", [P, (T // P) * ES], FP,
                                          isOutput=True)
        dbg_rs = nc.declare_dram_parameter("dbg_rs", [16, CH], FP,
                                           isOutput=True)
        dbg_ob = nc.declare_dram_parameter("dbg_ob", [P, B * HPC * L], FP,
                                           isOutput=True)
        dbg_xt = nc.declare_dram_parameter("dbg_xt", [P, N_DT * CH], FP,
                                           isOutput=True)
        dbg_q0 = nc.declare_dram_parameter("dbg_q0", [P, CH], FP,
                                           isOutput=True)

    xb = nc.dram_tensor("xb", [D, CH], BF)
    ag_xt = nc.dram_tensor("ag_xt", [N_CORES * D, CH], BF, addr_space="Shared")
    o_bounce, ag_o = {}, {}
    for b, pieces in AG_PIECES.items():
        for (c0, c1) in pieces:
            w = (c1 - c0) * CH
            o_bounce[(b, c0)] = nc.dram_tensor(f"o_bounce{b}_{c0}", [ES, w], BF)
            ag_o[(b, c0)] = nc.dram_tensor(f"ag_o{b}_{c0}", [N_CORES * ES, w], BF,
                                           addr_space="Shared")
    rg = [list(range(N_CORES))]

    with TileContext(nc, pool_alloc_mode="queue") as tc:
        with (
            tc.tile_pool(name="const", bufs=1) as const_pool,
            tc.tile_pool(name="resident", bufs=1) as res_pool,
            tc.tile_pool(name="wo", bufs=1) as wo_pool,
        ):
            ident = const_pool.tile([P, P], BF, name="ident")
            make_identity(nc, ident[:, :])
            trimask = const_pool.tile([P, P], FP, name="trimask")
            make_tri_mask(nc, trimask[:, :])
            ones_sb = const_pool.tile([P, 1], BF, name="ones_sb")
            nc.vector.memset(ones_sb[:, :], 1.0)
            ones_row = const_pool.tile([1, P], FP, name="ones_row")
            nc.vector.memset(ones_row[:, :], 1.0)

            # resident through phases 1-2
            qt_sb = res_pool.tile([P, HPC * T], BF, name="qt_sb")  # [hd, h*T+t]
            kt_sb = res_pool.tile([P, HPC * T], BF, name="kt_sb")
            v_sb = res_pool.tile([P, (T // P) * ES], BF, name="v_sb")  # [t%128, tt*ES+e]
            woT_sb = wo_pool.tile([P, N_DT * ES], BF, name="woT_sb")

            # ---- phase 0: transpose own 512-t block on the PE, AllGather
            with (
                tc.tile_pool(name="p0s", bufs=1) as p0s,
                tc.tile_pool(name="p0i", bufs=4) as p0i,
                tc.tile_pool(name="psT", bufs=2, space="PSUM") as psT,
            ):
                xts = p0s.tile([P, N_DT * CH], BF, name="xts")
                for tl in range(4):
                    for dt in range(N_DT):
                        xin = p0i.tile([P, P], BF, name="xin", tag="xin")
                        nc.sync.dma_start(
                            out=xin[:, :],
                            in_=x_c[dt * CH + tl * P: dt * CH + (tl + 1) * P,
                                    :])
                        txp = psT.tile([P, P], BF, name="txp", tag="tx")
                        nc.tensor.transpose(txp[:, :], xin[:, :], ident[:, :])
                        nc.vector.tensor_copy(
                            xts[:, dt * CH + tl * P: dt * CH + (tl + 1) * P],
                            txp[:, :])
                for dt in range(N_DT):
                    nc.sync.dma_start(out=xb[dt * P:(dt + 1) * P, :],
                                      in_=xts[:, dt * CH:(dt + 1) * CH])
                nc.gpsimd.collective_compute(
                    "AllGather", mybir.AluOpType.bypass,
                    ins=[xb[:]], outs=[ag_xt[:]], replica_groups=rg)

            # ---------------- phase 1: QKV^T + RoPE ----------------
            with (
                tc.tile_pool(name="wq", bufs=1) as wq_pool,
                tc.tile_pool(name="p1x", bufs=3) as p1x,
                tc.tile_pool(name="p1r", bufs=3) as p1r,
                tc.tile_pool(name="psQK", bufs=1, space="PSUM") as psQK,
                tc.tile_pool(name="psV", bufs=1, space="PSUM") as psV,
            ):
                wt_sb = wq_pool.tile([P, N_DT * 3 * ES], BF, name="wt_sb")
                c2_sb = wq_pool.tile([P, HPC * L], BF, name="c2_sb")
                s2_sb = wq_pool.tile([P, HPC * L], BF, name="s2_sb")

                for dt in range(N_DT):
                    nc.scalar.dma_start(
                        out=wt_sb[:, dt * 3 * ES:(dt + 1) * 3 * ES],
                        in_=wqkvT[dt * P:(dt + 1) * P, :])
                for h in range(HPC):
                    nc.scalar.dma_start(out=c2_sb[:, h * L:(h + 1) * L],
                                        in_=c2_p[h * P:(h + 1) * P, :])
                    nc.scalar.dma_start(out=s2_sb[:, h * L:(h + 1) * L],
                                        in_=s2_p[h * P:(h + 1) * P, :])

                last_dve = None
                dbg_holds = []
                chunk_last_mm = {}
                for ch in range(NCH):
                    xt = p1x.tile([P, N_DT * CH], BF, name="xt", tag="xt")
                    for dt in range(N_DT):
                        tp = nc.sync.dma_start(
                            out=xt[:, dt * CH:(dt + 1) * CH],
                            in_=ag_xt[ch * D + dt * P: ch * D + (dt + 1) * P,
                                      :])
                        if dt < 1:
                            if ch - 3 >= 0:
                                add_dep_helper(tp.ins, chunk_last_mm[ch - 3].ins,
                                               reason="xt slot WAR")
                            if debug and ch == 3 and dbg_holds:
                                add_dep_helper(tp.ins, dbg_holds[-1].ins,
                                               reason="xt dbg WAR")
                    qk_ps = {}
                    for part in range(2):
                        for h in range(HPC):
                            qk_ps[(part, h)] = psQK.tile(
                                [P, CH], FP, name="qk_ps", tag=f"qk{part}{h}")
                    v_ps = [psV.tile([P, ES], FP, name="v_ps", tag=f"v{i}")
                            for i in range(4)]
                    for dt in range(N_DT):
                        rhs = xt[:, dt * CH:(dt + 1) * CH]
                        for part in range(2):
                            for h in range(HPC):
                                mm = nc.tensor.matmul(
                                    qk_ps[(part, h)][:, :],
                                    wt_sb[:, dt * 3 * ES + (part * 2 + h) * P:
                                          dt * 3 * ES + (part * 2 + h + 1) * P],
                                    rhs, start=(dt == 0), stop=(dt == N_DT - 1))
                                if dt == 0 and part == 0 and h == 0 \
                                        and last_dve is not None:
                                    add_dep_helper(mm.ins, last_dve.ins,
                                                   reason="qk psum WAR")
                        for tl in range(4):
                            mm = nc.tensor.matmul(
                                v_ps[tl][:, :],
                                xt[:, dt * CH + tl * P: dt * CH + (tl + 1) * P],
                                wt_sb[:, dt * 3 * ES + 2 * ES:(dt + 1) * 3 * ES],
                                start=(dt == 0), stop=(dt == N_DT - 1))
                    chunk_last_mm[ch] = mm
                    # RoPE on q^T/k^T: rot = sb*C2 + swap(sb)*S2
                    if debug and ch == 0:
                        for dt in range(N_DT):
                            st = p1r.tile([P, CH], FP, name="dxt", tag="dxt")
                            nc.vector.tensor_copy(
                                st[:, :], xt[:, dt * CH:(dt + 1) * CH])
                            nc.sync.dma_start(
                                out=dbg_xt[:, dt * CH:(dt + 1) * CH],
                                in_=st[:, :])
                        stq = p1r.tile([P, CH], FP, name="dq0", tag="dq0")
                        dbg_holds.append(
                            nc.vector.tensor_copy(stq[:, :],
                                                  qk_ps[(0, 0)][:, :]))
                        nc.sync.dma_start(out=dbg_q0[:, :], in_=stq[:, :])
                    lq = (ch % (NCH // B)) * CH  # position within the sequence
                    for part in range(2):
                        for h in range(HPC):
                            ps = qk_ps[(part, h)]
                            m1 = p1r.tile([P, CH], BF, name="rm1", tag="rm1")
                            m2 = p1r.tile([P, CH], BF, name="rm2", tag="rm2")
                            ctab = c2_sb[:, h * L + lq: h * L + lq + CH]
                            stab = s2_sb[:, h * L + lq: h * L + lq + CH]
                            nc.vector.tensor_tensor(m1[:, :], ps[:, :], ctab,
                                                    op=mybir.AluOpType.mult)
                            # cross terms read the psum with swapped halves
                            nc.vector.tensor_tensor(
                                m2[0:64, :], ps[64:128, :], stab[0:64, :],
                                op=mybir.AluOpType.mult)
                            nc.vector.tensor_tensor(
                                m2[64:128, :], ps[0:64, :], stab[64:128, :],
                                op=mybir.AluOpType.mult)
                            dst = qt_sb if part == 0 else kt_sb
                            nc.vector.tensor_tensor(
                                dst[:, h * T + ch * CH: h * T + (ch + 1) * CH],
                                m1[:, :], m2[:, :], op=mybir.AluOpType.add)
                    for tl in range(4):
                        tt = ch * 4 + tl
                        last_dve = nc.vector.tensor_copy(
                            v_sb[:, tt * ES:(tt + 1) * ES], v_ps[tl][:, :])
                    if ch == 2:
                        for dt in range(N_DT):
                            nc.scalar.dma_start(
                                out=woT_sb[:, dt * ES:(dt + 1) * ES],
                                in_=woT[dt * P:(dt + 1) * P, :])

            # ---------------- phases 2+3 ----------------
            with (
                tc.tile_pool(name="p2p", bufs=8) as p2p,
                tc.tile_pool(name="p2sm", bufs=2) as p2sm,
                tc.tile_pool(name="p2ob", bufs=2) as p2ob,
                tc.tile_pool(name="p3x", bufs=2) as p3x,
                tc.tile_pool(name="p3o", bufs=2) as p3o,
                tc.tile_pool(name="psS", bufs=2, space="PSUM") as psS,
                tc.tile_pool(name="psO", bufs=2, space="PSUM") as psO,
                tc.tile_pool(name="psB", bufs=1, space="PSUM") as psB,
                tc.tile_pool(name="psF", bufs=1, space="PSUM") as psF,
            ):
                ob_copies = {}
                p2chain = {"exps": [], "norms": [], "bc_copy": None,
                           "f_copy": None}

                if debug:
                    for nm, src, dst in (("qt", qt_sb, dbg_qt),
                                         ("kt", kt_sb, dbg_kt),
                                         ("v", v_sb, dbg_v)):
                        for i in range(16):
                            st = p2sm.tile([P, CH], FP, name="dst",
                                           tag="dbgst")
                            nc.vector.tensor_copy(
                                st[:, :], src[:, i * CH:(i + 1) * CH])
                            nc.sync.dma_start(
                                out=dst[:, i * CH:(i + 1) * CH], in_=st[:, :])

                def phase2(b):
                    ob = p2ob.tile([P, HPC * L], BF, name="ob", tag="ob")
                    for qc in range(4):
                        for h in range(HPC):
                            qoff = h * T + b * L + qc * CH
                            koff = h * T + b * L
                            nblk = 4 * qc + 4
                            ngrp = nblk // 2
                            o_ps = psO.tile([P, CH], FP, name="o_ps", tag="o")
                            rs_ps = psB.tile([P, CH], FP, name="rs_ps",
                                             tag="bcrs")

                            def scores(g):
                                s_ps = psS.tile([P, 2 * CH], FP, name="s_ps",
                                                tag="s")
                                pt = p2p.tile([P, 2 * CH], BF, name="pt",
                                              tag="pt")
                                for j in range(2):
                                    kb = 2 * g + j
                                    jd = kb - 4 * qc
                                    off = jd * P if jd >= 0 else 0
                                    mm = nc.tensor.matmul(
                                        s_ps[:, j * CH + off:(j + 1) * CH],
                                        kt_sb[:, koff + kb * P:
                                              koff + (kb + 1) * P],
                                        qt_sb[:, qoff + off: qoff + CH],
                                        start=True, stop=True)
                                    if j == 0 and len(p2chain["exps"]) >= 2:
                                        add_dep_helper(
                                            mm.ins, p2chain["exps"][-2].ins,
                                            reason="s psum WAR")
                                    if jd >= 0:
                                        dsl = s_ps[:, j * CH + off:
                                                   j * CH + off + P]
                                        nc.vector.tensor_tensor(
                                            dsl, dsl, trimask[:, :],
                                            op=mybir.AluOpType.add)
                                ex = nc.scalar.activation(
                                    pt[:, :], s_ps[:, :],
                                    mybir.ActivationFunctionType.Exp,
                                    scale=SCALE)
                                p2chain["exps"].append(ex)
                                return pt

                            def pv(g, pt):
                                for j in range(2):
                                    kb = 2 * g + j
                                    jd = kb - 4 * qc
                                    off = jd * P if jd >= 0 else 0
                                    tt = b * (L // P) + kb
                                    mm = nc.tensor.matmul(
                                        o_ps[:, off:],
                                        v_sb[:, tt * ES + h * HD:
                                             tt * ES + (h + 1) * HD],
                                        pt[:, j * CH + off:(j + 1) * CH],
                                        start=(kb == 0), stop=(kb == nblk - 1))
                                    if kb == 0 and len(p2chain["norms"]) >= 2:
                                        add_dep_helper(
                                            mm.ins, p2chain["norms"][-2].ins,
                                            reason="o psum WAR")

                            pts = [scores(0)]
                            for g in range(1, ngrp):
                                pts.append(scores(g))
                                pv(g - 1, pts[g - 1])
                            pv(ngrp - 1, pts[ngrp - 1])
                            for g in range(ngrp):
                                for j in range(2):
                                    kb = 2 * g + j
                                    jd = kb - 4 * qc
                                    off = jd * P if jd >= 0 else 0
                                    mm = nc.tensor.matmul(
                                        rs_ps[0:1, off:], ones_sb[:, :],
                                        pts[g][:, j * CH + off:(j + 1) * CH],
                                        start=(kb == 0), stop=(kb == nblk - 1))
                                    if kb == 0 and p2chain["bc_copy"] is not None:
                                        add_dep_helper(
                                            mm.ins, p2chain["bc_copy"].ins,
                                            reason="rs psum WAR")

                            if debug:
                                rs_st = p2sm.tile([1, CH], FP, name="rs_st",
                                                  tag="rsst")
                                rsst_cp = nc.vector.tensor_copy(
                                    rs_st[:, :], rs_ps[0:1, :])
                                nc.sync.dma_start(
                                    out=dbg_rs[b * 8 + qc * 2 + h:
                                               b * 8 + qc * 2 + h + 1, :],
                                    in_=rs_st[:, :])
                            rs_sb = p2sm.tile([1, CH], FP, name="rs_sb",
                                              tag="rssb")
                            rscp = nc.vector.tensor_copy(rs_sb[:, :],
                                                         rs_ps[0:1, :])
                            rsp = p2sm.tile([P, 4], FP, name="rsp", tag="rsp")
                            nc.sync.dma_start(out=rsp[:, :], in_=rs_sb[0:1, :])
                            rcpp = p2sm.tile([P, 4], FP, name="rcpp", tag="rcpp")
                            nc.vector.reciprocal(rcpp[:, :], rsp[:, :])
                            rcp = p2sm.tile([1, CH], FP, name="rcp", tag="rcp")
                            nc.sync.dma_start(out=rcp[0:1, :], in_=rcpp[:, :])
                            bcmm = nc.tensor.matmul(
                                rs_ps[:, :], ones_row[:, :],
                                rcp[:, :], start=True, stop=True)
                            add_dep_helper(bcmm.ins, rscp.ins,
                                           reason="bc over rs WAR")
                            if debug:
                                add_dep_helper(bcmm.ins, rsst_cp.ins,
                                               reason="bc over rs dbg WAR")
                            bc = p2sm.tile([P, CH], FP, name="bc", tag="bc")
                            p2chain["bc_copy"] = nc.vector.tensor_copy(
                                bc[:, :], rs_ps[:, :])
                            obcp = nc.vector.tensor_tensor(
                                ob[:, h * L + qc * CH: h * L + (qc + 1) * CH],
                                o_ps[:, :], bc[:, :], op=mybir.AluOpType.mult)
                            ob_copies[(b, qc)] = obcp
                            p2chain["norms"].append(obcp)
                            if debug:
                                ob_st = p2sm.tile([P, CH], FP, name="ob_st",
                                                  tag="obst")
                                nc.vector.tensor_copy(
                                    ob_st[:, :],
                                    ob[:, h * L + qc * CH:
                                       h * L + (qc + 1) * CH])
                                nc.sync.dma_start(
                                    out=dbg_ob[:, (b * HPC + h) * L + qc * CH:
                                               (b * HPC + h) * L +
                                               (qc + 1) * CH],
                                    in_=ob_st[:, :])
                        for (c0, c1) in AG_PIECES[b]:
                            if c1 == qc + 1:
                                for h in range(HPC):
                                    nc.sync.dma_start(
                                        out=o_bounce[(b, c0)][h * HD:(h + 1) * HD, :],
                                        in_=ob[:, h * L + c0 * CH:
                                               h * L + c1 * CH])
                                nc.gpsimd.collective_compute(
                                    "AllGather", mybir.AluOpType.bypass,
                                    ins=[o_bounce[(b, c0)][:]],
                                    outs=[ag_o[(b, c0)][:]],
                                    replica_groups=rg)

                def phase3(b, c0, c1, dep=None):
                    w = (c1 - c0) * CH
                    for tch in range(w // CH):
                        ot = p3x.tile([P, N_DT * CH], BF, name="ot", tag="ot")
                        for dt in range(N_DT):
                            d = nc.sync.dma_start(
                                out=ot[:, dt * CH:(dt + 1) * CH],
                                in_=ag_o[(b, c0)][dt * P:(dt + 1) * P,
                                                  tch * CH:(tch + 1) * CH])
                            if dep is not None and tch == 0:
                                add_dep_helper(d.ins, dep.ins,
                                               reason="stagger ph3 behind AG")
                        t0 = b * L + (c0 + tch) * CH
                        for et in range(2):
                            f_ps = psF.tile([P, CH], FP, name="f_ps", tag="f")
                            for dt in range(N_DT):
                                mm = nc.tensor.matmul(
                                    f_ps[:, :],
                                    woT_sb[:, dt * ES + et * P:
                                           dt * ES + (et + 1) * P],
                                    ot[:, dt * CH:(dt + 1) * CH],
                                    start=(dt == 0), stop=(dt == N_DT - 1))
                                if dt == 0 and p2chain["f_copy"] is not None:
                                    add_dep_helper(
                                        mm.ins, p2chain["f_copy"].ins,
                                        reason="f psum WAR")
                            f_sb = p3o.tile([P, CH], FP, name="f_sb", tag="fsb")
                            p2chain["f_copy"] = nc.vector.tensor_copy(
                                f_sb[:, :], f_ps[:, :])
                            nc.sync.dma_start(
                                out=out[et * P:(et + 1) * P, t0:t0 + CH],
                                in_=f_sb[:, :])

                phase2(0)
                phase2(1)
                phase3(0, 0, 2, dep=ob_copies[(0, 3)])
                phase3(0, 2, 4, dep=ob_copies[(1, 0)])
                phase3(1, 0, 2, dep=ob_copies[(1, 2)])
                phase3(1, 2, 3, dep=ob_copies[(1, 3)])
                phase3(1, 3, 4)

    split_multi_waits(nc)
    return nc


def make_in_maps(x, cos, sin, Wqkv, Wo):
    bf = ml_dtypes.bfloat16
    xr = np.asarray(x).reshape(T, N_DT, P)  # [t, dt, d_lane]
    cosT = np.asarray(cos).T  # [D, L]
    sinT = np.asarray(sin).T
    eo = np.concatenate([2 * np.arange(64), 2 * np.arange(64) + 1])
    in_maps = []
    for c in range(N_CORES):
        blocks = []
        for part in range(2):  # q, k
            for h in range(HPC):
                g = c * HPC + h
                rows = part * D + g * HD + eo
                blocks.append(Wqkv[rows, :])
        wv = Wqkv[2 * D + c * ES: 2 * D + (c + 1) * ES, :]
        w_c = np.concatenate(blocks + [wv], axis=0)  # [768, D]
        c2s, s2s = [], []
        for h in range(HPC):
            g = c * HPC + h
            idx_e = g * HD + 2 * np.arange(64)
            c_h = cosT[idx_e, :]
            s_h = sinT[idx_e, :]
            c2s.append(np.concatenate([c_h, c_h], axis=0))
            s2s.append(np.concatenate([-s_h, s_h], axis=0))
        x_blk = np.ascontiguousarray(
            xr[c * CH:(c + 1) * CH].transpose(1, 0, 2).reshape(N_DT * CH, P)
        ).astype(bf)
        in_maps.append({
            "x_c": x_blk,
            "wqkvT": np.ascontiguousarray(w_c.T).astype(bf),
            "c2_p": np.ascontiguousarray(np.concatenate(c2s, 0)).astype(bf),
            "s2_p": np.ascontiguousarray(np.concatenate(s2s, 0)).astype(bf),
            "woT": np.ascontiguousarray(Wo[c * ES:(c + 1) * ES, :].T).astype(bf),
        })
    return in_maps


_cache = {}


def kernel(x, cos, sin, Wqkv, Wo):
    from concourse.bass_utils import run_bass_kernel_spmd
    x = np.asarray(x, dtype=np.float32)
    cos = np.asarray(cos, dtype=np.float32)
    sin = np.asarray(sin, dtype=np.float32)
    Wqkv = np.asarray(Wqkv, dtype=np.float32)
    Wo = np.asarray(Wo, dtype=np.float32)
    if "nc" not in _cache:
        _cache["nc"] = build()
    nc = _cache["nc"]
    in_maps = make_in_maps(x, cos, sin, Wqkv, Wo)
    res = run_bass_kernel_spmd(nc, in_maps, core_ids=list(range(N_CORES)))
    pieces = [res.results[c]["out"].T for c in range(N_CORES)]
    return np.concatenate(pieces, axis=1).reshape(B, L, D)


# revision 25
# speedup vs baseline: 1.1037x; 1.1037x over previous
"""Distributed causal attention block (QKV + RoPE + SDPA + Wo) on 8 TRN2 cores.

Tensor-parallel over heads (2 heads/core). Redesign vs baseline: no PE
transposes anywhere.

  phase 1: x^T tiles come from the DMA XBAR transpose engine; the QKV
           projection computes q^T/k^T directly ([head_dim, t] layout,
           lhsT = W^T tile) and v in [t, e] layout (lhsT = x^T tile).
           RoPE runs in the transposed layout: per head the 128 rows are
           [evens; odds] (host-permuted W columns), the e/o cross terms
           read the PSUM with swapped partition halves (mixed-space DVE
           ops), the sign of sin folded into the table: rot = q*C2 + sw*S2.
  phase 2: scores computed TRANSPOSED (s^T[k, q] = kt^T @ qt), exp on the
           scalar engine straight into bf16 P^T tiles (PV rhs), causal
           handled by a [128,128] transposed-triangle mask add on diagonal
           blocks plus column trimming. Softmax row sums via ones-column
           matmuls (M=1) accumulated in PSUM; normalization applied to the
           (tiny) attention output: o^T * broadcast(1/rowsum).
  phase 3: AllGather attention outputs (progressive pieces) -> Wo e-slice.
Host concatenates the 8 e-slices.
"""
import numpy as np
import ml_dtypes
import bass_rust
import concourse.bass as bass
import concourse.mybir as mybir
from concourse.tile import TileContext, add_dep_helper
from concourse.masks import make_identity

B, L, D, H = 2, 2048, 2048, 16
HD = 128
N_CORES = 8
HPC = H // N_CORES          # heads per core = 2
ES = HPC * HD               # 256 = e-slice width per core
T = B * L                   # 4096 tokens
P = 128
CH = 512                    # t-chunk
NCH = T // CH               # 8 chunks
N_DT = D // P               # 16 d-tiles
SCALE = 1.0 / float(np.sqrt(HD))
NEG = -30000.0              # causal fill; exp(SCALE*(s+NEG)) underflows to 0
FP = mybir.dt.float32
BF = mybir.dt.bfloat16

# attention-out AllGather pieces per batch, in units of 512-t q-chunks
AG_PIECES = {0: [(0, 2), (2, 4)], 1: [(0, 2), (2, 3), (3, 4)]}


def split_multi_waits(nc):
    """This walrus build allows 1 sync wait per instruction (2 for
    EventSemaphore). Tile attaches more on some instructions; hoist the
    extras onto same-engine NoOps."""
    for f in nc.m.functions:
        for bb in f.blocks:
            new_insts = []
            changed = False
            for ins in bb.instructions:
                si = ins.sync_info
                cap = 2 if type(ins).__name__ == "InstEventSemaphore" else 1
                if si is not None and len(si.on_wait) > cap:
                    waits = list(si.on_wait)
                    for k, w in enumerate(waits[cap:]):
                        new_insts.append(mybir.InstNoOp(
                            name=f"{ins.name}-wsplit{k}", ins=[], outs=[],
                            engine=ins.engine,
                            sync_info=bass_rust.SyncInfo(on_wait=[w], on_update=[]),
                        ))
                    ins.sync_info = bass_rust.SyncInfo(
                        on_wait=waits[:cap], on_update=list(si.on_update))
                    changed = True
                new_insts.append(ins)
            if changed:
                bb.instructions.clear()
                for i2 in new_insts:
                    bb.add_instruction(i2)


def make_tri_mask(nc, ap):
    """ap[k, q] = 0 where k <= q, NEG where k > q (transposed causal)."""
    nc.gpsimd.memset(ap, 0.0)
    nc.gpsimd.affine_select(
        out=ap, in_=ap,
        compare_op=mybir.AluOpType.is_ge,
        fill=NEG, base=0,
        # keep where (-1*x + 1*y) >= 0 i.e. q >= k
        pattern=[[1, ap.shape[1]]],
        channel_multiplier=-1,
    )


def build(debug=False):
    nc = bass.Bass()
    x_c = nc.declare_dram_parameter("x_c", [N_DT * CH, P], BF, isOutput=False)
    wqkvT = nc.declare_dram_parameter("wqkvT", [D, 3 * ES], BF, isOutput=False)
    c2_p = nc.declare_dram_parameter("c2_p", [HPC * P, L], BF, isOutput=False)
    s2_p = nc.declare_dram_parameter("s2_p", [HPC * P, L], BF, isOutput=False)
    woT = nc.declare_dram_parameter("woT", [D, ES], BF, isOutput=False)
    out = nc.declare_dram_parameter("out", [ES, T], FP, isOutput=True)
    if debug:
        dbg_qt = nc.declare_dram_parameter("dbg_qt", [P, HPC * T], FP,
                                           isOutput=True)
        dbg_kt = nc.declare_dram_parameter("dbg_kt", [P, HPC * T], FP,
                                           isOutput=True)
        dbg_v = nc.declare_dram_parameter("dbg_v", [P, (T // P) * ES], FP,
                                          isOutput=True)
        dbg_rs = nc.declare_dram_parameter("dbg_rs", [16, CH], FP,
                                           isOutput=True)
        dbg_ob = nc.declare_dram_parameter("dbg_ob", [P, B * HPC * L], FP,
                                           isOutput=True)
        dbg_xt = nc.declare_dram_parameter("dbg_xt", [P, N_DT * CH], FP,
                                           isOutput=True)
        dbg_q0 = nc.declare_dram_parameter("dbg_q0", [P, CH], FP,
                                           isOutput=True)

    xbq = [nc.dram_tensor(f"xb{q}", [D // 4, CH], BF) for q in range(4)]
    ag_xtq = [nc.dram_tensor(f"ag_xt{q}", [N_CORES * (D // 4), CH], BF,
                             addr_space="Shared") for q in range(4)]
    wup = nc.dram_tensor("wup", [1, 16], BF)
    ag_wup = nc.dram_tensor("ag_wup", [N_CORES, 16], BF, addr_space="Shared")
    o_bounce, ag_o = {}, {}
    for b, pieces in AG_PIECES.items():
        for (c0, c1) in pieces:
            w = (c1 - c0) * CH
            o_bounce[(b, c0)] = nc.dram_tensor(f"o_bounce{b}_{c0}", [ES, w], BF)
            ag_o[(b, c0)] = nc.dram_tensor(f"ag_o{b}_{c0}", [N_CORES * ES, w], BF,
                                           addr_space="Shared")
    rg = [list(range(N_CORES))]

    with TileContext(nc, pool_alloc_mode="queue") as tc:
        with (
            tc.tile_pool(name="const", bufs=1) as const_pool,
            tc.tile_pool(name="resident", bufs=1) as res_pool,
            tc.tile_pool(name="wo", bufs=1) as wo_pool,
        ):
            ident = const_pool.tile([P, P], BF, name="ident")
            make_identity(nc, ident[:, :])
            trimask = const_pool.tile([P, P], FP, name="trimask")
            make_tri_mask(nc, trimask[:, :])
            ones_sb = const_pool.tile([P, 1], BF, name="ones_sb")
            nc.vector.memset(ones_sb[:, :], 1.0)
            ones_row = const_pool.tile([1, P], FP, name="ones_row")
            nc.vector.memset(ones_row[:, :], 1.0)

            # resident through phases 1-2
            qt_sb = res_pool.tile([P, HPC * T], BF, name="qt_sb")  # [hd, h*T+t]
            kt_sb = res_pool.tile([P, HPC * T], BF, name="kt_sb")
            v_sb = res_pool.tile([P, (T // P) * ES], BF, name="v_sb")  # [t%128, tt*ES+e]
            woT_sb = wo_pool.tile([P, N_DT * ES], BF, name="woT_sb")

            # ---- phase 0: transpose own 512-t block on the PE, AllGather
            with (
                tc.tile_pool(name="p0s", bufs=1) as p0s,
                tc.tile_pool(name="p0i", bufs=4) as p0i,
                tc.tile_pool(name="psT", bufs=2, space="PSUM") as psT,
            ):
                wup_sb = p0s.tile([1, 16], BF, name="wup_sb")
                nc.vector.memset(wup_sb[:, :], 0.0)
                nc.sync.dma_start(out=wup[:, :], in_=wup_sb[:, :])
                nc.gpsimd.collective_compute(
                    "AllGather", mybir.AluOpType.bypass,
                    ins=[wup[:]], outs=[ag_wup[:]], replica_groups=rg)
                xts = p0s.tile([P, N_DT * CH], BF, name="xts")
                for q in range(4):
                    for dt in range(q * 4, q * 4 + 4):
                        for tl in range(4):
                            xin = p0i.tile([P, P], BF, name="xin", tag="xin")
                            nc.sync.dma_start(
                                out=xin[:, :],
                                in_=x_c[dt * CH + tl * P:
                                        dt * CH + (tl + 1) * P, :])
                            txp = psT.tile([P, P], BF, name="txp", tag="tx")
                            nc.tensor.transpose(txp[:, :], xin[:, :],
                                                ident[:, :])
                            nc.vector.tensor_copy(
                                xts[:, dt * CH + tl * P:
                                    dt * CH + (tl + 1) * P],
                                txp[:, :])
                        nc.sync.dma_start(
                            out=xbq[q][(dt - q * 4) * P:
                                       (dt - q * 4 + 1) * P, :],
                            in_=xts[:, dt * CH:(dt + 1) * CH])
                    nc.gpsimd.collective_compute(
                        "AllGather", mybir.AluOpType.bypass,
                        ins=[xbq[q][:]], outs=[ag_xtq[q][:]],
                        replica_groups=rg)

            # ---------------- phase 1: QKV^T + RoPE ----------------
            with (
                tc.tile_pool(name="wq", bufs=1) as wq_pool,
                tc.tile_pool(name="p1x", bufs=3) as p1x,
                tc.tile_pool(name="p1r", bufs=3) as p1r,
                tc.tile_pool(name="psQK", bufs=3, space="PSUM") as psQK,
                tc.tile_pool(name="psV", bufs=3, space="PSUM") as psV,
            ):
                wt_sb = wq_pool.tile([P, N_DT * 3 * ES], BF, name="wt_sb")
                c2_sb = wq_pool.tile([P, HPC * L], BF, name="c2_sb")
                s2_sb = wq_pool.tile([P, HPC * L], BF, name="s2_sb")

                for dt in range(N_DT):
                    nc.scalar.dma_start(
                        out=wt_sb[:, dt * 3 * ES:(dt + 1) * 3 * ES],
                        in_=wqkvT[dt * P:(dt + 1) * P, :])
                for h in range(HPC):
                    nc.scalar.dma_start(out=c2_sb[:, h * L:(h + 1) * L],
                                        in_=c2_p[h * P:(h + 1) * P, :])
                    nc.scalar.dma_start(out=s2_sb[:, h * L:(h + 1) * L],
                                        in_=s2_p[h * P:(h + 1) * P, :])

                qk_readers, v_readers = [], []
                dbg_holds = []
                chunk_last_mm = {}
                for ch in range(NCH):
                    xt = p1x.tile([P, N_DT * CH], BF, name="xt", tag="xt")
                    for dt in range(N_DT):
                        q, dl = dt // 4, dt % 4
                        tp = nc.sync.dma_start(
                            out=xt[:, dt * CH:(dt + 1) * CH],
                            in_=ag_xtq[q][ch * (D // 4) + dl * P:
                                          ch * (D // 4) + (dl + 1) * P, :])
                        if dt < 1:
                            if ch - 3 >= 0:
                                add_dep_helper(tp.ins, chunk_last_mm[ch - 3].ins,
                                               reason="xt slot WAR")
                            if debug and ch == 3 and dbg_holds:
                                add_dep_helper(tp.ins, dbg_holds[-1].ins,
                                               reason="xt dbg WAR")
                    lq = (ch % (NCH // B)) * CH  # position within the sequence
                    for part in range(2):
                        for h in range(HPC):
                            ps = psQK.tile([P, CH], FP, name="qk_ps", tag="qk")
                            for dt in range(N_DT):
                                mm = nc.tensor.matmul(
                                    ps[:, :],
                                    wt_sb[:, dt * 3 * ES + (part * 2 + h) * P:
                                          dt * 3 * ES + (part * 2 + h + 1) * P],
                                    xt[:, dt * CH:(dt + 1) * CH],
                                    start=(dt == 0), stop=(dt == N_DT - 1))
                                if dt == 0 and len(qk_readers) >= 3:
                                    add_dep_helper(mm.ins, qk_readers[-3].ins,
                                                   reason="qk psum WAR")
                            if debug and ch == 0 and part == 0 and h == 0:
                                for dt in range(N_DT):
                                    st = p1r.tile([P, CH], FP, name="dxt",
                                                  tag="dxt")
                                    nc.vector.tensor_copy(
                                        st[:, :], xt[:, dt * CH:(dt + 1) * CH])
                                    nc.sync.dma_start(
                                        out=dbg_xt[:, dt * CH:(dt + 1) * CH],
                                        in_=st[:, :])
                                stq = p1r.tile([P, CH], FP, name="dq0",
                                               tag="dq0")
                                dbg_holds.append(
                                    nc.vector.tensor_copy(stq[:, :], ps[:, :]))
                                nc.sync.dma_start(out=dbg_q0[:, :],
                                                  in_=stq[:, :])
                            m1 = p1r.tile([P, CH], BF, name="rm1", tag="rm1")
                            m2 = p1r.tile([P, CH], BF, name="rm2", tag="rm2")
                            ctab = c2_sb[:, h * L + lq: h * L + lq + CH]
                            stab = s2_sb[:, h * L + lq: h * L + lq + CH]
                            nc.vector.tensor_tensor(m1[:, :], ps[:, :], ctab,
                                                    op=mybir.AluOpType.mult)
                            # cross terms read the psum with swapped halves
                            nc.vector.tensor_tensor(
                                m2[0:64, :], ps[64:128, :], stab[0:64, :],
                                op=mybir.AluOpType.mult)
                            mr = nc.vector.tensor_tensor(
                                m2[64:128, :], ps[0:64, :], stab[64:128, :],
                                op=mybir.AluOpType.mult)
                            qk_readers.append(mr)
                            dst = qt_sb if part == 0 else kt_sb
                            nc.vector.tensor_tensor(
                                dst[:, h * T + ch * CH: h * T + (ch + 1) * CH],
                                m1[:, :], m2[:, :], op=mybir.AluOpType.add)
                    for tl in range(4):
                        vps = psV.tile([P, ES], FP, name="v_ps", tag="v")
                        for dt in range(N_DT):
                            mm = nc.tensor.matmul(
                                vps[:, :],
                                xt[:, dt * CH + tl * P: dt * CH + (tl + 1) * P],
                                wt_sb[:, dt * 3 * ES + 2 * ES:
                                      (dt + 1) * 3 * ES],
                                start=(dt == 0), stop=(dt == N_DT - 1))
                            if dt == 0 and len(v_readers) >= 3:
                                add_dep_helper(mm.ins, v_readers[-3].ins,
                                               reason="v psum WAR")
                        tt = ch * 4 + tl
                        v_readers.append(nc.vector.tensor_copy(
                            v_sb[:, tt * ES:(tt + 1) * ES], vps[:, :]))
                    chunk_last_mm[ch] = mm
                    if ch == 2:
                        for dt in range(N_DT):
                            nc.scalar.dma_start(
                                out=woT_sb[:, dt * ES:(dt + 1) * ES],
                                in_=woT[dt * P:(dt + 1) * P, :])

            # ---------------- phases 2+3 ----------------
            with (
                tc.tile_pool(name="p2p", bufs=8) as p2p,
                tc.tile_pool(name="p2sm", bufs=2) as p2sm,
                tc.tile_pool(name="p2ob", bufs=2) as p2ob,
                tc.tile_pool(name="p3x", bufs=2) as p3x,
                tc.tile_pool(name="p3o", bufs=2) as p3o,
                tc.tile_pool(name="psS", bufs=2, space="PSUM") as psS,
                tc.tile_pool(name="psO", bufs=2, space="PSUM") as psO,
                tc.tile_pool(name="psB", bufs=1, space="PSUM") as psB,
                tc.tile_pool(name="psF", bufs=1, space="PSUM") as psF,
            ):
                ob_copies = {}
                p2chain = {"exps": [], "norms": [], "bc_copy": None,
                           "f_copy": None}

                if debug:
                    for nm, src, dst in (("qt", qt_sb, dbg_qt),
                                         ("kt", kt_sb, dbg_kt),
                                         ("v", v_sb, dbg_v)):
                        for i in range(16):
                            st = p2sm.tile([P, CH], FP, name="dst",
                                           tag="dbgst")
                            nc.vector.tensor_copy(
                                st[:, :], src[:, i * CH:(i + 1) * CH])
                            nc.sync.dma_start(
                                out=dst[:, i * CH:(i + 1) * CH], in_=st[:, :])

                def phase2(b):
                    ob = p2ob.tile([P, HPC * L], BF, name="ob", tag="ob")
                    for qc in range(4):
                        for h in range(HPC):
                            qoff = h * T + b * L + qc * CH
                            koff = h * T + b * L
                            nblk = 4 * qc + 4
                            ngrp = nblk // 2
                            o_ps = psO.tile([P, CH], FP, name="o_ps", tag="o")
                            rs_ps = psB.tile([P, CH], FP, name="rs_ps",
                                             tag="bcrs")

                            def scores(g):
                                s_ps = psS.tile([P, 2 * CH], FP, name="s_ps",
                                                tag="s")
                                pt = p2p.tile([P, 2 * CH], BF, name="pt",
                                              tag="pt")
                                for j in range(2):
                                    kb = 2 * g + j
                                    jd = kb - 4 * qc
                                    off = jd * P if jd >= 0 else 0
                                    mm = nc.tensor.matmul(
                                        s_ps[:, j * CH + off:(j + 1) * CH],
                                        kt_sb[:, koff + kb * P:
                                              koff + (kb + 1) * P],
                                        qt_sb[:, qoff + off: qoff + CH],
                                        start=True, stop=True)
                                    if j == 0 and len(p2chain["exps"]) >= 2:
                                        add_dep_helper(
                                            mm.ins, p2chain["exps"][-2].ins,
                                            reason="s psum WAR")
                                    if jd >= 0:
                                        dsl = s_ps[:, j * CH + off:
                                                   j * CH + off + P]
                                        nc.vector.tensor_tensor(
                                            dsl, dsl, trimask[:, :],
                                            op=mybir.AluOpType.add)
                                ex = nc.scalar.activation(
                                    pt[:, :], s_ps[:, :],
                                    mybir.ActivationFunctionType.Exp,
                                    scale=SCALE)
                                p2chain["exps"].append(ex)
                                return pt

                            def pv(g, pt):
                                for j in range(2):
                                    kb = 2 * g + j
                                    jd = kb - 4 * qc
                                    off = jd * P if jd >= 0 else 0
                                    tt = b * (L // P) + kb
                                    mm = nc.tensor.matmul(
                                        o_ps[:, off:],
                                        v_sb[:, tt * ES + h * HD:
                                             tt * ES + (h + 1) * HD],
                                        pt[:, j * CH + off:(j + 1) * CH],
                                        start=(kb == 0), stop=(kb == nblk - 1))
                                    if kb == 0 and len(p2chain["norms"]) >= 2:
                                        add_dep_helper(
                                            mm.ins, p2chain["norms"][-2].ins,
                                            reason="o psum WAR")

                            pts = [scores(0)]
                            for g in range(1, ngrp):
                                pts.append(scores(g))
                                pv(g - 1, pts[g - 1])
                            pv(ngrp - 1, pts[ngrp - 1])
                            for g in range(ngrp):
                                for j in range(2):
                                    kb = 2 * g + j
                                    jd = kb - 4 * qc
                                    off = jd * P if jd >= 0 else 0
                                    mm = nc.tensor.matmul(
                                        rs_ps[0:1, off:], ones_sb[:, :],
                                        pts[g][:, j * CH + off:(j + 1) * CH],
                                        start=(kb == 0), stop=(kb == nblk - 1))
                                    if kb == 0 and p2chain["bc_copy"] is not None:
                                        add_dep_helper(
                                            mm.ins, p2chain["bc_copy"].ins,
                                            reason="rs psum WAR")

                            if debug:
                                rs_st = p2sm.tile([1, CH], FP, name="rs_st",
                                                  tag="rsst")
                                rsst_cp = nc.vector.tensor_copy(
                                    rs_st[:, :], rs_ps[0:1, :])
                                nc.sync.dma_start(
                                    out=dbg_rs[b * 8 + qc * 2 + h:
                                               b * 8 + qc * 2 + h + 1, :],
                                    in_=rs_st[:, :])
                            rs_sb = p2sm.tile([1, CH], FP, name="rs_sb",
                                              tag="rssb")
                            rscp = nc.vector.tensor_copy(rs_sb[:, :],
                                                         rs_ps[0:1, :])
                            rsp = p2sm.tile([P, 4], FP, name="rsp", tag="rsp")
                            nc.sync.dma_start(out=rsp[:, :], in_=rs_sb[0:1, :])
                            rcpp = p2sm.tile([P, 4], FP, name="rcpp", tag="rcpp")
                            nc.vector.reciprocal(rcpp[:, :], rsp[:, :])
                            rcp = p2sm.tile([1, CH], FP, name="rcp", tag="rcp")
                            nc.sync.dma_start(out=rcp[0:1, :], in_=rcpp[:, :])
                            bcmm = nc.tensor.matmul(
                                rs_ps[:, :], ones_row[:, :],
                                rcp[:, :], start=True, stop=True)
                            add_dep_helper(bcmm.ins, rscp.ins,
                                           reason="bc over rs WAR")
                            if debug:
                                add_dep_helper(bcmm.ins, rsst_cp.ins,
                                               reason="bc over rs dbg WAR")
                            bc = p2sm.tile([P, CH], FP, name="bc", tag="bc")
                            p2chain["bc_copy"] = nc.vector.tensor_copy(
                                bc[:, :], rs_ps[:, :])
                            obcp = nc.vector.tensor_tensor(
                                ob[:, h * L + qc * CH: h * L + (qc + 1) * CH],
                                o_ps[:, :], bc[:, :], op=mybir.AluOpType.mult)
                            ob_copies[(b, qc)] = obcp
                            p2chain["norms"].append(obcp)
                            if debug:
                                ob_st = p2sm.tile([P, CH], FP, name="ob_st",
                                                  tag="obst")
                                nc.vector.tensor_copy(
                                    ob_st[:, :],
                                    ob[:, h * L + qc * CH:
                                       h * L + (qc + 1) * CH])
                                nc.sync.dma_start(
                                    out=dbg_ob[:, (b * HPC + h) * L + qc * CH:
                                               (b * HPC + h) * L +
                                               (qc + 1) * CH],
                                    in_=ob_st[:, :])
                        for (c0, c1) in AG_PIECES[b]:
                            if c1 == qc + 1:
                                for h in range(HPC):
                                    nc.sync.dma_start(
                                        out=o_bounce[(b, c0)][h * HD:(h + 1) * HD, :],
                                        in_=ob[:, h * L + c0 * CH:
                                               h * L + c1 * CH])
                                nc.gpsimd.collective_compute(
                                    "AllGather", mybir.AluOpType.bypass,
                                    ins=[o_bounce[(b, c0)][:]],
                                    outs=[ag_o[(b, c0)][:]],
                                    replica_groups=rg)

                def phase3(b, c0, c1, dep=None):
                    w = (c1 - c0) * CH
                    for tch in range(w // CH):
                        ot = p3x.tile([P, N_DT * CH], BF, name="ot", tag="ot")
                        for dt in range(N_DT):
                            d = nc.sync.dma_start(
                                out=ot[:, dt * CH:(dt + 1) * CH],
                                in_=ag_o[(b, c0)][dt * P:(dt + 1) * P,
                                                  tch * CH:(tch + 1) * CH])
                            if dep is not None and tch == 0:
                                add_dep_helper(d.ins, dep.ins,
                                               reason="stagger ph3 behind AG")
                        t0 = b * L + (c0 + tch) * CH
                        for et in range(2):
                            f_ps = psF.tile([P, CH], FP, name="f_ps", tag="f")
                            for dt in range(N_DT):
                                mm = nc.tensor.matmul(
                                    f_ps[:, :],
                                    woT_sb[:, dt * ES + et * P:
                                           dt * ES + (et + 1) * P],
                                    ot[:, dt * CH:(dt + 1) * CH],
                                    start=(dt == 0), stop=(dt == N_DT - 1))
                                if dt == 0 and p2chain["f_copy"] is not None:
                                    add_dep_helper(
                                        mm.ins, p2chain["f_copy"].ins,
                                        reason="f psum WAR")
                            f_sb = p3o.tile([P, CH], FP, name="f_sb", tag="fsb")
                            p2chain["f_copy"] = nc.vector.tensor_copy(
                                f_sb[:, :], f_ps[:, :])
                            nc.sync.dma_start(
                                out=out[et * P:(et + 1) * P, t0:t0 + CH],
                                in_=f_sb[:, :])

                phase2(0)
                phase2(1)
                phase3(0, 0, 2, dep=ob_copies[(0, 3)])
                phase3(0, 2, 4, dep=ob_copies[(1, 0)])
                phase3(1, 0, 2, dep=ob_copies[(1, 2)])
                phase3(1, 2, 3, dep=ob_copies[(1, 3)])
                phase3(1, 3, 4)

    split_multi_waits(nc)
    return nc


def make_in_maps(x, cos, sin, Wqkv, Wo):
    bf = ml_dtypes.bfloat16
    xr = np.asarray(x).reshape(T, N_DT, P)  # [t, dt, d_lane]
    cosT = np.asarray(cos).T  # [D, L]
    sinT = np.asarray(sin).T
    eo = np.concatenate([2 * np.arange(64), 2 * np.arange(64) + 1])
    in_maps = []
    for c in range(N_CORES):
        blocks = []
        for part in range(2):  # q, k
            for h in range(HPC):
                g = c * HPC + h
                rows = part * D + g * HD + eo
                blocks.append(Wqkv[rows, :])
        wv = Wqkv[2 * D + c * ES: 2 * D + (c + 1) * ES, :]
        w_c = np.concatenate(blocks + [wv], axis=0)  # [768, D]
        c2s, s2s = [], []
        for h in range(HPC):
            g = c * HPC + h
            idx_e = g * HD + 2 * np.arange(64)
            c_h = cosT[idx_e, :]
            s_h = sinT[idx_e, :]
            c2s.append(np.concatenate([c_h, c_h], axis=0))
            s2s.append(np.concatenate([-s_h, s_h], axis=0))
        x_blk = np.ascontiguousarray(
            xr[c * CH:(c + 1) * CH].transpose(1, 0, 2).reshape(N_DT * CH, P)
        ).astype(bf)
        in_maps.append({
            "x_c": x_blk,
            "wqkvT": np.ascontiguousarray(w_c.T).astype(bf),
            "c2_p": np.ascontiguousarray(np.concatenate(c2s, 0)).astype(bf),
            "s2_p": np.ascontiguousarray(np.concatenate(s2s, 0)).astype(bf),
            "woT": np.ascontiguousarray(Wo[c * ES:(c + 1) * ES, :].T).astype(bf),
        })
    return in_maps


_cache = {}


def kernel(x, cos, sin, Wqkv, Wo):
    from concourse.bass_utils import run_bass_kernel_spmd
    x = np.asarray(x, dtype=np.float32)
    cos = np.asarray(cos, dtype=np.float32)
    sin = np.asarray(sin, dtype=np.float32)
    Wqkv = np.asarray(Wqkv, dtype=np.float32)
    Wo = np.asarray(Wo, dtype=np.float32)
    if "nc" not in _cache:
        _cache["nc"] = build()
    nc = _cache["nc"]
    in_maps = make_in_maps(x, cos, sin, Wqkv, Wo)
    res = run_bass_kernel_spmd(nc, in_maps, core_ids=list(range(N_CORES)))
    pieces = [res.results[c]["out"].T for c in range(N_CORES)]
    return np.concatenate(pieces, axis=1).reshape(B, L, D)


# revision 27
# speedup vs baseline: 1.1096x; 1.0054x over previous
"""Distributed causal attention block (QKV + RoPE + SDPA + Wo) on 8 TRN2 cores.

Tensor-parallel over heads (2 heads/core). Redesign vs baseline: no PE
transposes anywhere.

  phase 1: x^T tiles come from the DMA XBAR transpose engine; the QKV
           projection computes q^T/k^T directly ([head_dim, t] layout,
           lhsT = W^T tile) and v in [t, e] layout (lhsT = x^T tile).
           RoPE runs in the transposed layout: per head the 128 rows are
           [evens; odds] (host-permuted W columns), the e/o cross terms
           read the PSUM with swapped partition halves (mixed-space DVE
           ops), the sign of sin folded into the table: rot = q*C2 + sw*S2.
  phase 2: scores computed TRANSPOSED (s^T[k, q] = kt^T @ qt), exp on the
           scalar engine straight into bf16 P^T tiles (PV rhs), causal
           handled by a [128,128] transposed-triangle mask add on diagonal
           blocks plus column trimming. Softmax row sums via ones-column
           matmuls (M=1) accumulated in PSUM; normalization applied to the
           (tiny) attention output: o^T * broadcast(1/rowsum).
  phase 3: AllGather attention outputs (progressive pieces) -> Wo e-slice.
Host concatenates the 8 e-slices.
"""
import numpy as np
import ml_dtypes
import bass_rust
import concourse.bass as bass
import concourse.mybir as mybir
from concourse.tile import TileContext, add_dep_helper
from concourse.masks import make_identity

B, L, D, H = 2, 2048, 2048, 16
HD = 128
N_CORES = 8
HPC = H // N_CORES          # heads per core = 2
ES = HPC * HD               # 256 = e-slice width per core
T = B * L                   # 4096 tokens
P = 128
CH = 512                    # t-chunk
NCH = T // CH               # 8 chunks
N_DT = D // P               # 16 d-tiles
SCALE = 1.0 / float(np.sqrt(HD))
NEG = -30000.0              # causal fill; exp(SCALE*(s+NEG)) underflows to 0
FP = mybir.dt.float32
BF = mybir.dt.bfloat16

# attention-out AllGather pieces per batch, in units of 512-t q-chunks
AG_PIECES = {0: [(0, 2), (2, 4)], 1: [(0, 2), (2, 4)]}


def split_multi_waits(nc):
    """This walrus build allows 1 sync wait per instruction (2 for
    EventSemaphore). Tile attaches more on some instructions; hoist the
    extras onto same-engine NoOps."""
    for f in nc.m.functions:
        for bb in f.blocks:
            new_insts = []
            changed = False
            for ins in bb.instructions:
                si = ins.sync_info
                cap = 2 if type(ins).__name__ == "InstEventSemaphore" else 1
                if si is not None and len(si.on_wait) > cap:
                    waits = list(si.on_wait)
                    for k, w in enumerate(waits[cap:]):
                        new_insts.append(mybir.InstNoOp(
                            name=f"{ins.name}-wsplit{k}", ins=[], outs=[],
                            engine=ins.engine,
                            sync_info=bass_rust.SyncInfo(on_wait=[w], on_update=[]),
                        ))
                    ins.sync_info = bass_rust.SyncInfo(
                        on_wait=waits[:cap], on_update=list(si.on_update))
                    changed = True
                new_insts.append(ins)
            if changed:
                bb.instructions.clear()
                for i2 in new_insts:
                    bb.add_instruction(i2)


def make_tri_mask(nc, ap):
    """ap[k, q] = 0 where k <= q, NEG where k > q (transposed causal)."""
    nc.gpsimd.memset(ap, 0.0)
    nc.gpsimd.affine_select(
        out=ap, in_=ap,
        compare_op=mybir.AluOpType.is_ge,
        fill=NEG, base=0,
        # keep where (-1*x + 1*y) >= 0 i.e. q >= k
        pattern=[[1, ap.shape[1]]],
        channel_multiplier=-1,
    )


def build(debug=False):
    nc = bass.Bass()
    x_c = nc.declare_dram_parameter("x_c", [N_DT * CH, P], BF, isOutput=False)
    wqkvT = nc.declare_dram_parameter("wqkvT", [D, 3 * ES], BF, isOutput=False)
    c2_p = nc.declare_dram_parameter("c2_p", [HPC * P, L], BF, isOutput=False)
    s2_p = nc.declare_dram_parameter("s2_p", [HPC * P, L], BF, isOutput=False)
    woT = nc.declare_dram_parameter("woT", [D, ES], BF, isOutput=False)
    out = nc.declare_dram_parameter("out", [ES, T], FP, isOutput=True)
    if debug:
        dbg_qt = nc.declare_dram_parameter("dbg_qt", [P, HPC * T], FP,
                                           isOutput=True)
        dbg_kt = nc.declare_dram_parameter("dbg_kt", [P, HPC * T], FP,
                                           isOutput=True)
        dbg_v = nc.declare_dram_parameter("dbg_v", [P, (T // P) * ES], FP,
                                          isOutput=True)
        dbg_rs = nc.declare_dram_parameter("dbg_rs", [16, CH], FP,
                                           isOutput=True)
        dbg_ob = nc.declare_dram_parameter("dbg_ob", [P, B * HPC * L], FP,
                                           isOutput=True)
        dbg_xt = nc.declare_dram_parameter("dbg_xt", [P, N_DT * CH], FP,
                                           isOutput=True)
        dbg_q0 = nc.declare_dram_parameter("dbg_q0", [P, CH], FP,
                                           isOutput=True)

    xb = nc.dram_tensor("xb", [D, CH], BF)
    ag_xt = nc.dram_tensor("ag_xt", [N_CORES * D, CH], BF, addr_space="Shared")
    wup = nc.dram_tensor("wup", [1, 16], BF)
    ag_wup = nc.dram_tensor("ag_wup", [N_CORES, 16], BF, addr_space="Shared")
    o_bounce, ag_o = {}, {}
    for b, pieces in AG_PIECES.items():
        for (c0, c1) in pieces:
            w = (c1 - c0) * CH
            o_bounce[(b, c0)] = nc.dram_tensor(f"o_bounce{b}_{c0}", [ES, w], BF)
            ag_o[(b, c0)] = nc.dram_tensor(f"ag_o{b}_{c0}", [N_CORES * ES, w], BF,
                                           addr_space="Shared")
    rg = [list(range(N_CORES))]

    with TileContext(nc, pool_alloc_mode="queue") as tc:
        with (
            tc.tile_pool(name="const", bufs=1) as const_pool,
            tc.tile_pool(name="resident", bufs=1) as res_pool,
            tc.tile_pool(name="wo", bufs=1) as wo_pool,
        ):
            ident = const_pool.tile([P, P], BF, name="ident")
            make_identity(nc, ident[:, :])
            trimask = const_pool.tile([P, P], FP, name="trimask")
            make_tri_mask(nc, trimask[:, :])
            ones_sb = const_pool.tile([P, 1], BF, name="ones_sb")
            nc.vector.memset(ones_sb[:, :], 1.0)
            ones_row = const_pool.tile([1, P], FP, name="ones_row")
            nc.vector.memset(ones_row[:, :], 1.0)

            # resident through phases 1-2
            qt_sb = res_pool.tile([P, HPC * T], BF, name="qt_sb")  # [hd, h*T+t]
            kt_sb = res_pool.tile([P, HPC * T], BF, name="kt_sb")
            v_sb = res_pool.tile([P, (T // P) * ES], BF, name="v_sb")  # [t%128, tt*ES+e]
            woT_sb = wo_pool.tile([P, N_DT * ES], BF, name="woT_sb")

            # ---- phase 0: transpose own 512-t block on the PE, AllGather
            with (
                tc.tile_pool(name="p0s", bufs=1) as p0s,
                tc.tile_pool(name="p0i", bufs=4) as p0i,
                tc.tile_pool(name="psT", bufs=2, space="PSUM") as psT,
            ):
                wup_sb = p0s.tile([1, 16], BF, name="wup_sb")
                nc.vector.memset(wup_sb[:, :], 0.0)
                nc.sync.dma_start(out=wup[:, :], in_=wup_sb[:, :])
                nc.gpsimd.collective_compute(
                    "AllGather", mybir.AluOpType.bypass,
                    ins=[wup[:]], outs=[ag_wup[:]], replica_groups=rg)
                xts = p0s.tile([P, N_DT * CH], BF, name="xts")
                for dt in range(N_DT):
                    for tl in range(4):
                        xin = p0i.tile([P, P], BF, name="xin", tag="xin")
                        nc.sync.dma_start(
                            out=xin[:, :],
                            in_=x_c[dt * CH + tl * P:
                                    dt * CH + (tl + 1) * P, :])
                        txp = psT.tile([P, P], BF, name="txp", tag="tx")
                        nc.tensor.transpose(txp[:, :], xin[:, :], ident[:, :])
                        nc.vector.tensor_copy(
                            xts[:, dt * CH + tl * P: dt * CH + (tl + 1) * P],
                            txp[:, :])
                    nc.sync.dma_start(out=xb[dt * P:(dt + 1) * P, :],
                                      in_=xts[:, dt * CH:(dt + 1) * CH])
                nc.gpsimd.collective_compute(
                    "AllGather", mybir.AluOpType.bypass,
                    ins=[xb[:]], outs=[ag_xt[:]], replica_groups=rg)

            # ---------------- phase 1: QKV^T + RoPE ----------------
            with (
                tc.tile_pool(name="wq", bufs=1) as wq_pool,
                tc.tile_pool(name="p1x", bufs=3) as p1x,
                tc.tile_pool(name="p1r", bufs=3) as p1r,
                tc.tile_pool(name="psQK", bufs=3, space="PSUM") as psQK,
                tc.tile_pool(name="psV", bufs=3, space="PSUM") as psV,
            ):
                wt_sb = wq_pool.tile([P, N_DT * 3 * ES], BF, name="wt_sb")
                c2_sb = wq_pool.tile([P, HPC * L], BF, name="c2_sb")
                s2_sb = wq_pool.tile([P, HPC * L], BF, name="s2_sb")

                for dt in range(N_DT):
                    nc.scalar.dma_start(
                        out=wt_sb[:, dt * 3 * ES:(dt + 1) * 3 * ES],
                        in_=wqkvT[dt * P:(dt + 1) * P, :])
                for h in range(HPC):
                    nc.scalar.dma_start(out=c2_sb[:, h * L:(h + 1) * L],
                                        in_=c2_p[h * P:(h + 1) * P, :])
                    nc.scalar.dma_start(out=s2_sb[:, h * L:(h + 1) * L],
                                        in_=s2_p[h * P:(h + 1) * P, :])

                qk_readers, v_readers = [], []
                dbg_holds = []
                chunk_last_mm = {}
                for ch in range(NCH):
                    xt = p1x.tile([P, N_DT * CH], BF, name="xt", tag="xt")
                    for dt in range(N_DT):
                        tp = nc.sync.dma_start(
                            out=xt[:, dt * CH:(dt + 1) * CH],
                            in_=ag_xt[ch * D + dt * P:
                                      ch * D + (dt + 1) * P, :])
                        if dt < 1:
                            if ch - 3 >= 0:
                                add_dep_helper(tp.ins, chunk_last_mm[ch - 3].ins,
                                               reason="xt slot WAR")
                            if debug and ch == 3 and dbg_holds:
                                add_dep_helper(tp.ins, dbg_holds[-1].ins,
                                               reason="xt dbg WAR")
                    lq = (ch % (NCH // B)) * CH  # position within the sequence
                    for part in range(2):
                        for h in range(HPC):
                            ps = psQK.tile([P, CH], FP, name="qk_ps", tag="qk")
                            for dt in range(N_DT):
                                mm = nc.tensor.matmul(
                                    ps[:, :],
                                    wt_sb[:, dt * 3 * ES + (part * 2 + h) * P:
                                          dt * 3 * ES + (part * 2 + h + 1) * P],
                                    xt[:, dt * CH:(dt + 1) * CH],
                                    start=(dt == 0), stop=(dt == N_DT - 1))
                                if dt == 0 and len(qk_readers) >= 3:
                                    add_dep_helper(mm.ins, qk_readers[-3].ins,
                                                   reason="qk psum WAR")
                            if debug and ch == 0 and part == 0 and h == 0:
                                for dt in range(N_DT):
                                    st = p1r.tile([P, CH], FP, name="dxt",
                                                  tag="dxt")
                                    nc.vector.tensor_copy(
                                        st[:, :], xt[:, dt * CH:(dt + 1) * CH])
                                    nc.sync.dma_start(
                                        out=dbg_xt[:, dt * CH:(dt + 1) * CH],
                                        in_=st[:, :])
                                stq = p1r.tile([P, CH], FP, name="dq0",
                                               tag="dq0")
                                dbg_holds.append(
                                    nc.vector.tensor_copy(stq[:, :], ps[:, :]))
                                nc.sync.dma_start(out=dbg_q0[:, :],
                                                  in_=stq[:, :])
                            m1 = p1r.tile([P, CH], BF, name="rm1", tag="rm1")
                            m2 = p1r.tile([P, CH], BF, name="rm2", tag="rm2")
                            ctab = c2_sb[:, h * L + lq: h * L + lq + CH]
                            stab = s2_sb[:, h * L + lq: h * L + lq + CH]
                            nc.vector.tensor_tensor(m1[:, :], ps[:, :], ctab,
                                                    op=mybir.AluOpType.mult)
                            # cross terms read the psum with swapped halves
                            nc.vector.tensor_tensor(
                                m2[0:64, :], ps[64:128, :], stab[0:64, :],
                                op=mybir.AluOpType.mult)
                            mr = nc.vector.tensor_tensor(
                                m2[64:128, :], ps[0:64, :], stab[64:128, :],
                                op=mybir.AluOpType.mult)
                            qk_readers.append(mr)
                            dst = qt_sb if part == 0 else kt_sb
                            nc.vector.tensor_tensor(
                                dst[:, h * T + ch * CH: h * T + (ch + 1) * CH],
                                m1[:, :], m2[:, :], op=mybir.AluOpType.add)
                    for tl in range(4):
                        vps = psV.tile([P, ES], FP, name="v_ps", tag="v")
                        for dt in range(N_DT):
                            mm = nc.tensor.matmul(
                                vps[:, :],
                                xt[:, dt * CH + tl * P: dt * CH + (tl + 1) * P],
                                wt_sb[:, dt * 3 * ES + 2 * ES:
                                      (dt + 1) * 3 * ES],
                                start=(dt == 0), stop=(dt == N_DT - 1))
                            if dt == 0 and len(v_readers) >= 3:
                                add_dep_helper(mm.ins, v_readers[-3].ins,
                                               reason="v psum WAR")
                        tt = ch * 4 + tl
                        v_readers.append(nc.vector.tensor_copy(
                            v_sb[:, tt * ES:(tt + 1) * ES], vps[:, :]))
                    chunk_last_mm[ch] = mm
                    if ch == 2:
                        for dt in range(N_DT):
                            nc.scalar.dma_start(
                                out=woT_sb[:, dt * ES:(dt + 1) * ES],
                                in_=woT[dt * P:(dt + 1) * P, :])

            # ---------------- phases 2+3 ----------------
            with (
                tc.tile_pool(name="p2p", bufs=8) as p2p,
                tc.tile_pool(name="p2sm", bufs=2) as p2sm,
                tc.tile_pool(name="p2ob", bufs=2) as p2ob,
                tc.tile_pool(name="p3x", bufs=2) as p3x,
                tc.tile_pool(name="p3o", bufs=2) as p3o,
                tc.tile_pool(name="psS", bufs=2, space="PSUM") as psS,
                tc.tile_pool(name="psO", bufs=2, space="PSUM") as psO,
                tc.tile_pool(name="psB", bufs=1, space="PSUM") as psB,
                tc.tile_pool(name="psF", bufs=1, space="PSUM") as psF,
            ):
                ob_copies = {}
                p2chain = {"exps": [], "norms": [], "bc_copy": None,
                           "f_copy": None}

                if debug:
                    for nm, src, dst in (("qt", qt_sb, dbg_qt),
                                         ("kt", kt_sb, dbg_kt),
                                         ("v", v_sb, dbg_v)):
                        for i in range(16):
                            st = p2sm.tile([P, CH], FP, name="dst",
                                           tag="dbgst")
                            nc.vector.tensor_copy(
                                st[:, :], src[:, i * CH:(i + 1) * CH])
                            nc.sync.dma_start(
                                out=dst[:, i * CH:(i + 1) * CH], in_=st[:, :])

                def phase2(b):
                    ob = p2ob.tile([P, HPC * L], BF, name="ob", tag="ob")
                    for qc in range(4):
                        for h in range(HPC):
                            qoff = h * T + b * L + qc * CH
                            koff = h * T + b * L
                            nblk = 4 * qc + 4
                            ngrp = nblk // 2
                            o_ps = psO.tile([P, CH], FP, name="o_ps", tag="o")
                            rs_ps = psB.tile([1, CH], FP, name="rs_ps",
                                             tag="bcrs")

                            def scores(g):
                                s_ps = psS.tile([P, 2 * CH], FP, name="s_ps",
                                                tag="s")
                                pt = p2p.tile([P, 2 * CH], BF, name="pt",
                                              tag="pt")
                                for j in range(2):
                                    kb = 2 * g + j
                                    jd = kb - 4 * qc
                                    off = jd * P if jd >= 0 else 0
                                    mm = nc.tensor.matmul(
                                        s_ps[:, j * CH + off:(j + 1) * CH],
                                        kt_sb[:, koff + kb * P:
                                              koff + (kb + 1) * P],
                                        qt_sb[:, qoff + off: qoff + CH],
                                        start=True, stop=True)
                                    if j == 0 and len(p2chain["exps"]) >= 2:
                                        add_dep_helper(
                                            mm.ins, p2chain["exps"][-2].ins,
                                            reason="s psum WAR")
                                    if jd >= 0:
                                        dsl = s_ps[:, j * CH + off:
                                                   j * CH + off + P]
                                        nc.vector.tensor_tensor(
                                            dsl, dsl, trimask[:, :],
                                            op=mybir.AluOpType.add)
                                ex = nc.scalar.activation(
                                    pt[:, :], s_ps[:, :],
                                    mybir.ActivationFunctionType.Exp,
                                    scale=SCALE)
                                p2chain["exps"].append(ex)
                                return pt

                            def pv(g, pt):
                                for j in range(2):
                                    kb = 2 * g + j
                                    jd = kb - 4 * qc
                                    off = jd * P if jd >= 0 else 0
                                    tt = b * (L // P) + kb
                                    mm = nc.tensor.matmul(
                                        o_ps[:, off:],
                                        v_sb[:, tt * ES + h * HD:
                                             tt * ES + (h + 1) * HD],
                                        pt[:, j * CH + off:(j + 1) * CH],
                                        start=(kb == 0), stop=(kb == nblk - 1))
                                    if kb == 0 and len(p2chain["norms"]) >= 2:
                                        add_dep_helper(
                                            mm.ins, p2chain["norms"][-2].ins,
                                            reason="o psum WAR")

                            pts = [scores(0)]
                            for g in range(1, ngrp):
                                pts.append(scores(g))
                                pv(g - 1, pts[g - 1])
                            pv(ngrp - 1, pts[ngrp - 1])
                            for g in range(ngrp):
                                for j in range(2):
                                    kb = 2 * g + j
                                    jd = kb - 4 * qc
                                    off = jd * P if jd >= 0 else 0
                                    mm = nc.tensor.matmul(
                                        rs_ps[0:1, off:], ones_sb[:, :],
                                        pts[g][:, j * CH + off:(j + 1) * CH],
                                        start=(kb == 0), stop=(kb == nblk - 1))
                                    if kb == 0 and p2chain["bc_copy"] is not None:
                                        add_dep_helper(
                                            mm.ins, p2chain["bc_copy"].ins,
                                            reason="rs psum WAR")

                            if debug:
                                rs_st = p2sm.tile([1, CH], FP, name="rs_st",
                                                  tag="rsst")
                                rsst_cp = nc.vector.tensor_copy(
                                    rs_st[:, :], rs_ps[0:1, :])
                                nc.sync.dma_start(
                                    out=dbg_rs[b * 8 + qc * 2 + h:
                                               b * 8 + qc * 2 + h + 1, :],
                                    in_=rs_st[:, :])
                            rs_sb = p2sm.tile([1, CH], FP, name="rs_sb",
                                              tag="rssb")
                            rscp = nc.vector.tensor_copy(rs_sb[:, :],
                                                         rs_ps[0:1, :])
                            rsp = p2sm.tile([P, 4], FP, name="rsp", tag="rsp")
                            nc.sync.dma_start(out=rsp[:, :], in_=rs_sb[0:1, :])
                            rcpp = p2sm.tile([P, 4], FP, name="rcpp", tag="rcpp")
                            nc.vector.reciprocal(rcpp[:, :], rsp[:, :])
                            rcp = p2sm.tile([1, CH], FP, name="rcp", tag="rcp")
                            nc.sync.dma_start(out=rcp[0:1, :], in_=rcpp[:, :])
                            bc = p2sm.tile([P, CH], FP, name="bc", tag="bc")
                            nc.sync.dma_start(
                                out=bc[:, :],
                                in_=rcp[0:1, :].partition_broadcast(P))
                            p2chain["bc_copy"] = rscp
                            obcp = nc.vector.tensor_tensor(
                                ob[:, h * L + qc * CH: h * L + (qc + 1) * CH],
                                o_ps[:, :], bc[:, :], op=mybir.AluOpType.mult)
                            ob_copies[(b, qc)] = obcp
                            p2chain["norms"].append(obcp)
                            if debug:
                                ob_st = p2sm.tile([P, CH], FP, name="ob_st",
                                                  tag="obst")
                                nc.vector.tensor_copy(
                                    ob_st[:, :],
                                    ob[:, h * L + qc * CH:
                                       h * L + (qc + 1) * CH])
                                nc.sync.dma_start(
                                    out=dbg_ob[:, (b * HPC + h) * L + qc * CH:
                                               (b * HPC + h) * L +
                                               (qc + 1) * CH],
                                    in_=ob_st[:, :])
                        for (c0, c1) in AG_PIECES[b]:
                            if c1 == qc + 1:
                                for h in range(HPC):
                                    nc.sync.dma_start(
                                        out=o_bounce[(b, c0)][h * HD:(h + 1) * HD, :],
                                        in_=ob[:, h * L + c0 * CH:
                                               h * L + c1 * CH])
                                nc.gpsimd.collective_compute(
                                    "AllGather", mybir.AluOpType.bypass,
                                    ins=[o_bounce[(b, c0)][:]],
                                    outs=[ag_o[(b, c0)][:]],
                                    replica_groups=rg)

                def phase3(b, c0, c1, dep=None):
                    w = (c1 - c0) * CH
                    for tch in range(w // CH):
                        ot = p3x.tile([P, N_DT * CH], BF, name="ot", tag="ot")
                        for dt in range(N_DT):
                            d = nc.sync.dma_start(
                                out=ot[:, dt * CH:(dt + 1) * CH],
                                in_=ag_o[(b, c0)][dt * P:(dt + 1) * P,
                                                  tch * CH:(tch + 1) * CH])
                            if dep is not None and tch == 0:
                                add_dep_helper(d.ins, dep.ins,
                                               reason="stagger ph3 behind AG")
                        t0 = b * L + (c0 + tch) * CH
                        for et in range(2):
                            f_ps = psF.tile([P, CH], FP, name="f_ps", tag="f")
                            for dt in range(N_DT):
                                mm = nc.tensor.matmul(
                                    f_ps[:, :],
                                    woT_sb[:, dt * ES + et * P:
                                           dt * ES + (et + 1) * P],
                                    ot[:, dt * CH:(dt + 1) * CH],
                                    start=(dt == 0), stop=(dt == N_DT - 1))
                                if dt == 0 and p2chain["f_copy"] is not None:
                                    add_dep_helper(
                                        mm.ins, p2chain["f_copy"].ins,
                                        reason="f psum WAR")
                            f_sb = p3o.tile([P, CH], FP, name="f_sb", tag="fsb")
                            p2chain["f_copy"] = nc.vector.tensor_copy(
                                f_sb[:, :], f_ps[:, :])
                            nc.sync.dma_start(
                                out=out[et * P:(et + 1) * P, t0:t0 + CH],
                                in_=f_sb[:, :])

                phase2(0)
                phase2(1)
                phase3(0, 0, 2, dep=ob_copies[(0, 3)])
                phase3(0, 2, 4, dep=ob_copies[(1, 0)])
                phase3(1, 0, 2, dep=ob_copies[(1, 2)])
                phase3(1, 2, 4)

    split_multi_waits(nc)
    return nc


def make_in_maps(x, cos, sin, Wqkv, Wo):
    bf = ml_dtypes.bfloat16
    xr = np.asarray(x).reshape(T, N_DT, P)  # [t, dt, d_lane]
    cosT = np.asarray(cos).T  # [D, L]
    sinT = np.asarray(sin).T
    eo = np.concatenate([2 * np.arange(64), 2 * np.arange(64) + 1])
    in_maps = []
    for c in range(N_CORES):
        blocks = []
        for part in range(2):  # q, k
            for h in range(HPC):
                g = c * HPC + h
                rows = part * D + g * HD + eo
                blocks.append(Wqkv[rows, :])
        wv = Wqkv[2 * D + c * ES: 2 * D + (c + 1) * ES, :]
        w_c = np.concatenate(blocks + [wv], axis=0)  # [768, D]
        c2s, s2s = [], []
        for h in range(HPC):
            g = c * HPC + h
            idx_e = g * HD + 2 * np.arange(64)
            c_h = cosT[idx_e, :]
            s_h = sinT[idx_e, :]
            c2s.append(np.concatenate([c_h, c_h], axis=0))
            s2s.append(np.concatenate([-s_h, s_h], axis=0))
        x_blk = np.ascontiguousarray(
            xr[c * CH:(c + 1) * CH].transpose(1, 0, 2).reshape(N_DT * CH, P)
        ).astype(bf)
        in_maps.append({
            "x_c": x_blk,
            "wqkvT": np.ascontiguousarray(w_c.T).astype(bf),
            "c2_p": np.ascontiguousarray(np.concatenate(c2s, 0)).astype(bf),
            "s2_p": np.ascontiguousarray(np.concatenate(s2s, 0)).astype(bf),
            "woT": np.ascontiguousarray(Wo[c * ES:(c + 1) * ES, :].T).astype(bf),
        })
    return in_maps


_cache = {}


def kernel(x, cos, sin, Wqkv, Wo):
    from concourse.bass_utils import run_bass_kernel_spmd
    x = np.asarray(x, dtype=np.float32)
    cos = np.asarray(cos, dtype=np.float32)
    sin = np.asarray(sin, dtype=np.float32)
    Wqkv = np.asarray(Wqkv, dtype=np.float32)
    Wo = np.asarray(Wo, dtype=np.float32)
    if "nc" not in _cache:
        _cache["nc"] = build()
    nc = _cache["nc"]
    in_maps = make_in_maps(x, cos, sin, Wqkv, Wo)
    res = run_bass_kernel_spmd(nc, in_maps, core_ids=list(range(N_CORES)))
    pieces = [res.results[c]["out"].T for c in range(N_CORES)]
    return np.concatenate(pieces, axis=1).reshape(B, L, D)


# revision 32
# speedup vs baseline: 1.1342x; 1.0221x over previous
"""Distributed causal attention block (QKV + RoPE + SDPA + Wo) on 8 TRN2 cores.

Sharding: tensor-parallel over heads (2 heads/core). Each core:
  phase 1: streams full x (bf16), PE-transposes tiles inline, QKV projection
           for its 2 heads + RoPE + transposes -> QT/KT/V resident in SBUF
  phase 2: causal attention per (batch, head), q-chunk-grouped PV with the
           softmax normalization folded into a P-prescale (on GpSimd)
  phase 3: AllGather attention outputs (d-sharded, in progressive t-pieces,
           tapering at the end) -> Wo e-slice -> output
Host concatenates the 8 e-slices.

The q/k columns of Wqkv (and the cos/sin tables) are permuted head-major
even/odd on the host so RoPE runs on contiguous blocks; attention scores are
invariant to a shared permutation of the head dim of Q and K.
"""
import numpy as np
import ml_dtypes
import bass_rust
import concourse.bass as bass
import concourse.mybir as mybir
from concourse.tile import TileContext, add_dep_helper
from concourse.masks import make_identity, make_causal_mask

B, L, D, H = 2, 2048, 2048, 16
HD = 128
N_CORES = 8
HPC = H // N_CORES          # heads per core = 2
ES = HPC * HD               # 256 = e-slice width per core
T = B * L                   # 4096 tokens total
TS = T // N_CORES           # 512 t per rank-block in phase 1
P = 128
SCALE = 1.0 / float(np.sqrt(HD))
NEG = -30000.0              # causal mask fill; exp(SCALE*(s+NEG)) underflows to 0
FP = mybir.dt.float32
BF = mybir.dt.bfloat16

N_TT = T // P               # 32 global t-tiles
N_LT = L // P               # 16 t-tiles per batch
N_DT = D // P               # 16 d-tiles

# attention-out AllGather pieces per batch, in units of 512-t q-chunks (4/batch)
AG_PIECES = {0: [(0, 2), (2, 4)], 1: [(0, 2), (2, 3), (3, 4)]}


def split_multi_waits(nc):
    """This walrus build allows 1 sync wait per instruction (2 for
    EventSemaphore). Tile attaches more on some instructions (tail drain,
    collective-adjacent DMAs); hoist the extras onto same-engine NoOps."""
    for f in nc.m.functions:
        for bb in f.blocks:
            new_insts = []
            changed = False
            for ins in bb.instructions:
                si = ins.sync_info
                cap = 2 if type(ins).__name__ == "InstEventSemaphore" else 1
                if si is not None and len(si.on_wait) > cap:
                    waits = list(si.on_wait)
                    for k, w in enumerate(waits[cap:]):
                        new_insts.append(mybir.InstNoOp(
                            name=f"{ins.name}-wsplit{k}", ins=[], outs=[],
                            engine=ins.engine,
                            sync_info=bass_rust.SyncInfo(on_wait=[w], on_update=[]),
                        ))
                    ins.sync_info = bass_rust.SyncInfo(
                        on_wait=waits[:cap], on_update=list(si.on_update))
                    changed = True
                new_insts.append(ins)
            if changed:
                bb.instructions.clear()
                for i2 in new_insts:
                    bb.add_instruction(i2)


def build(debug=False):
    nc = bass.Bass()
    x_c = nc.declare_dram_parameter("x_c", [TS, D], BF, isOutput=False)
    xb = nc.dram_tensor("xb", [D, TS], BF)
    ag_xt = nc.dram_tensor("ag_xt", [N_CORES * D, TS], BF, addr_space="Shared")
    wup = nc.dram_tensor("wup", [1, 16], BF)
    ag_wup = nc.dram_tensor("ag_wup", [N_CORES, 16], BF, addr_space="Shared")
    wqkvT = nc.declare_dram_parameter("wqkvT", [D, 3 * ES], BF, isOutput=False)
    ce_p = nc.declare_dram_parameter("ce_p", [L, P], FP, isOutput=False)
    co_p = nc.declare_dram_parameter("co_p", [L, P], FP, isOutput=False)
    se_p = nc.declare_dram_parameter("se_p", [L, P], FP, isOutput=False)
    so_p = nc.declare_dram_parameter("so_p", [L, P], FP, isOutput=False)
    woT = nc.declare_dram_parameter("woT", [D, ES], BF, isOutput=False)
    out = nc.declare_dram_parameter("out", [ES, T], FP, isOutput=True)
    if debug:
        dbg_qt = nc.declare_dram_parameter("dbg_qt", [P, HPC * T], FP, isOutput=True)
        dbg_kt = nc.declare_dram_parameter("dbg_kt", [P, HPC * T], FP, isOutput=True)
        dbg_v = nc.declare_dram_parameter("dbg_v", [T, ES], FP, isOutput=True)
        dbg_o = nc.declare_dram_parameter("dbg_o", [ES, T], FP, isOutput=True)

    # out AllGather bounce/result per (batch, piece)
    o_bounce, ag_o = {}, {}
    for b, pieces in AG_PIECES.items():
        for (c0, c1) in pieces:
            w = (c1 - c0) * 512
            o_bounce[(b, c0)] = nc.dram_tensor(f"o_bounce{b}_{c0}", [ES, w], BF)
            ag_o[(b, c0)] = nc.dram_tensor(f"ag_o{b}_{c0}", [N_CORES * ES, w], BF,
                                           addr_space="Shared")
    rg = [list(range(N_CORES))]

    def r3(ap):  # [128, 256] -> [128, 2 heads, 2 (even/odd), 64]
        return ap.rearrange("p (h s x) -> p h s x", h=2, s=2)

    def r2(ap):  # [128, 128] -> [128, 2 heads, 64]
        return ap.rearrange("p (h x) -> p h x", h=2)

    with TileContext(nc, pool_alloc_mode="queue") as tc:
        with (
            tc.tile_pool(name="const", bufs=1) as const_pool,
            tc.tile_pool(name="resident", bufs=1) as res_pool,
            tc.tile_pool(name="wo", bufs=1) as wo_pool,
            tc.tile_pool(name="psA", bufs=2, space="PSUM") as psA,
            tc.tile_pool(name="psB", bufs=2, space="PSUM") as psB,
            tc.tile_pool(name="psC", bufs=2, space="PSUM") as psC,
            tc.tile_pool(name="psD", bufs=2, space="PSUM") as psD,
        ):
            ident = const_pool.tile([P, P], BF, name="ident")
            make_identity(nc, ident[:, :])
            cmask = const_pool.tile([P, P], FP, name="cmask")
            make_causal_mask(nc, cmask[:, :], mask_val=NEG)

            # resident through phases 1-2
            qt_sb = res_pool.tile([P, HPC * T], BF, name="qt_sb")   # [hd', h*T + t]
            kt_sb = res_pool.tile([P, HPC * T], BF, name="kt_sb")
            v_sb = res_pool.tile([P, N_TT * ES], BF, name="v_sb")   # [t%128, tt*ES+e]

            # ---------------- phase 1: x^T tiles + QKV + RoPE ----------------
            with (
                tc.tile_pool(name="wq", bufs=1) as wq_pool,
                tc.tile_pool(name="p1n", bufs=4) as p1n,
                tc.tile_pool(name="p1x", bufs=2) as p1x,
                tc.tile_pool(name="p1t", bufs=3) as p1t,
            ):
                wt_sb = wq_pool.tile([P, N_DT * 3 * ES], BF, name="wt_sb")
                trig_sb = {}
                for nm in ("ce", "co", "se", "so"):
                    trig_sb[nm] = wq_pool.tile([P, N_LT * P], FP, name=f"{nm}_sb")
                woT_sb = wo_pool.tile([P, N_DT * ES], BF, name="woT_sb")

                # warm-up collective: absorbs the one-time CC barrier
                wup_sb = p1t.tile([1, 16], BF, name="wup_sb", tag="wup")
                nc.vector.memset(wup_sb[:, :], 0.0)
                nc.sync.dma_start(out=wup[:, :], in_=wup_sb[:, :])
                nc.gpsimd.collective_compute(
                    "AllGather", mybir.AluOpType.bypass,
                    ins=[wup[:]], outs=[ag_wup[:]], replica_groups=rg)

                # phase 0: transpose only our own 512-t block, AllGather x^T
                xins0 = []
                for tl in range(TS // P):
                    xin = p1n.tile([P, D], BF, name="xin", tag="xin")
                    nc.sync.dma_start(out=xin[:, :],
                                      in_=x_c[tl * P:(tl + 1) * P, :])
                    xins0.append(xin)
                for dt in range(N_DT):
                    nc.sync.dma_start(out=wt_sb[:, dt * 3 * ES:(dt + 1) * 3 * ES],
                                      in_=wqkvT[dt * P:(dt + 1) * P, :])
                for nm, prm in (("ce", ce_p), ("co", co_p),
                                ("se", se_p), ("so", so_p)):
                    for lt in range(N_LT):
                        nc.sync.dma_start(out=trig_sb[nm][:, lt * P:(lt + 1) * P],
                                          in_=prm[lt * P:(lt + 1) * P, :])
                xts = p1x.tile([P, N_DT * TS], BF, name="xts")
                for dt in range(N_DT):
                    txp = psC.tile([P, TS], BF, name="txp", tag="C")
                    for tl in range(TS // P):
                        nc.tensor.transpose(
                            txp[:, tl * P:(tl + 1) * P],
                            xins0[tl][:, dt * P:(dt + 1) * P], ident[:, :])
                    nc.any.tensor_copy(xts[:, dt * TS:(dt + 1) * TS],
                                       txp[:, :])
                    nc.scalar.dma_start(out=xb[dt * P:(dt + 1) * P, :],
                                        in_=xts[:, dt * TS:(dt + 1) * TS])
                nc.gpsimd.collective_compute(
                    "AllGather", mybir.AluOpType.bypass,
                    ins=[xb[:]], outs=[ag_xt[:]], replica_groups=rg)

                for rb in range(N_CORES):
                    xt_rb = p1x.tile([P, N_DT * TS], BF, name="xt_rb")
                    for dt in range(N_DT):
                        nc.sync.dma_start(
                            out=xt_rb[:, dt * TS:(dt + 1) * TS],
                            in_=ag_xt[rb * D + dt * P: rb * D + (dt + 1) * P,
                                      :])
                    if rb == N_CORES - 1:
                        for dt in range(N_DT):
                            nc.sync.dma_start(
                                out=woT_sb[:, dt * ES:(dt + 1) * ES],
                                in_=woT[dt * P:(dt + 1) * P, :])
                    for tl in range(TS // P):
                        tt = rb * (TS // P) + tl
                        lt = tt % N_LT
                        ps_qk = psA.tile([P, 2 * ES], FP, name="ps_qk", tag="A")
                        ps_v = psB.tile([P, ES], FP, name="ps_v", tag="B")
                        for dt in range(N_DT):
                            lhsT = xt_rb[:, dt * TS + tl * P: dt * TS + (tl + 1) * P]
                            nc.tensor.matmul(
                                ps_qk[:, :], lhsT,
                                wt_sb[:, dt * 3 * ES: dt * 3 * ES + 2 * ES],
                                start=(dt == 0), stop=(dt == N_DT - 1))
                            nc.tensor.matmul(
                                ps_v[:, :], lhsT,
                                wt_sb[:, dt * 3 * ES + 2 * ES:(dt + 1) * 3 * ES],
                                start=(dt == 0), stop=(dt == N_DT - 1))
                        nc.vector.tensor_copy(v_sb[:, tt * ES:(tt + 1) * ES],
                                              ps_v[:, :])
                        ce = r2(trig_sb["ce"][:, lt * P:(lt + 1) * P])
                        co = r2(trig_sb["co"][:, lt * P:(lt + 1) * P])
                        se = r2(trig_sb["se"][:, lt * P:(lt + 1) * P])
                        so = r2(trig_sb["so"][:, lt * P:(lt + 1) * P])
                        for part in range(2):  # 0=q, 1=k
                            src = r3(ps_qk[:, part * ES:(part + 1) * ES])
                            e_, o_ = src[:, :, 0, :], src[:, :, 1, :]
                            rot = p1t.tile([P, ES], BF, name="rot", tag="rot")
                            rdst = r3(rot[:, :])
                            re_, ro_ = rdst[:, :, 0, :], rdst[:, :, 1, :]
                            t1 = p1t.tile([P, P], FP, name="t1", tag="t1")
                            t2 = p1t.tile([P, P], FP, name="t2", tag="t2")
                            t13, t23 = r2(t1[:, :]), r2(t2[:, :])
                            nc.vector.tensor_tensor(t13, e_, ce,
                                                    op=mybir.AluOpType.mult)
                            nc.vector.tensor_tensor(t23, o_, se,
                                                    op=mybir.AluOpType.mult)
                            nc.vector.tensor_tensor(re_, t13, t23,
                                                    op=mybir.AluOpType.subtract)
                            nc.vector.tensor_tensor(t13, o_, co,
                                                    op=mybir.AluOpType.mult)
                            nc.vector.tensor_tensor(t23, e_, so,
                                                    op=mybir.AluOpType.mult)
                            nc.vector.tensor_tensor(ro_, t13, t23,
                                                    op=mybir.AluOpType.add)
                            dst = qt_sb if part == 0 else kt_sb
                            for h in range(HPC):
                                tps = psD.tile([P, P], BF, name="tps", tag="D")
                                nc.tensor.transpose(
                                    tps[:, :], rot[:, h * HD:(h + 1) * HD],
                                    ident[:, :])
                                nc.vector.tensor_copy(
                                    dst[:, h * T + tt * P: h * T + (tt + 1) * P],
                                    tps[:, :])

            if debug:
                with tc.tile_pool(name="dbgp", bufs=2) as dbgp:
                    for nm, src in (("dbg_qt", qt_sb), ("dbg_kt", kt_sb)):
                        dd = {"dbg_qt": dbg_qt, "dbg_kt": dbg_kt}[nm]
                        for i in range(HPC * T // 512):
                            s = dbgp.tile([P, 512], FP, name="dstage")
                            nc.vector.tensor_copy(s[:, :],
                                                  src[:, i * 512:(i + 1) * 512])
                            nc.sync.dma_start(out=dd[:, i * 512:(i + 1) * 512],
                                              in_=s[:, :])
                    for tt in range(N_TT):
                        s = dbgp.tile([P, ES], FP, name="dstage2")
                        nc.vector.tensor_copy(s[:, :], v_sb[:, tt * ES:(tt + 1) * ES])
                        nc.sync.dma_start(out=dbg_v[tt * P:(tt + 1) * P, :],
                                          in_=s[:, :])

            # ---------------- phases 2+3 (shared pools, interleaved) ----------
            with (
                tc.tile_pool(name="p2p", bufs=6) as p2p,
                tc.tile_pool(name="p2pt", bufs=3) as p2pt,
                tc.tile_pool(name="p2sm", bufs=4) as p2sm,
                tc.tile_pool(name="p2ob", bufs=2) as p2ob,
                tc.tile_pool(name="p3x", bufs=2) as p3x,
                tc.tile_pool(name="p3o", bufs=2) as p3o,
            ):
                ob_copies = {}

                def phase2(b):
                    ob_sb = p2ob.tile([P, HPC * L], BF, name="ob_sb", tag="ob")
                    for qc in range(4):
                        for h in range(HPC):
                            qoff = h * T + b * L
                            psbs = []
                            for qi in range(qc * 4, qc * 4 + 4):
                                kend = (qi + 1) * P
                                nch = (kend + 511) // 512
                                p_sb = p2p.tile([P, L], BF, name="p_sb", tag="p")
                                sums = p2sm.tile([P, 4], FP, name="sums", tag="sums")
                                for ci in range(nch):
                                    klo = ci * 512
                                    ksz = min(512, kend - klo)
                                    s_ps = psA.tile([P, 512], FP, name="s_ps", tag="A")
                                    nc.tensor.matmul(
                                        s_ps[:, :ksz],
                                        qt_sb[:, qoff + qi * P: qoff + (qi + 1) * P],
                                        kt_sb[:, qoff + klo: qoff + klo + ksz],
                                        start=True, stop=True)
                                    if klo + ksz == kend:  # diagonal 128-block
                                        dslice = s_ps[:, ksz - P:ksz]
                                        nc.vector.tensor_tensor(
                                            dslice, dslice, cmask[:, :],
                                            op=mybir.AluOpType.add)
                                    nc.scalar.activation(
                                        p_sb[:, klo:klo + ksz], s_ps[:, :ksz],
                                        mybir.ActivationFunctionType.Exp,
                                        scale=SCALE,
                                        accum_out=sums[:, ci:ci + 1])
                                tot = p2sm.tile([P, 1], FP, name="tot", tag="tot")
                                if nch > 1:
                                    nc.vector.tensor_reduce(
                                        tot[:, :], sums[:, :nch],
                                        axis=mybir.AxisListType.X,
                                        op=mybir.AluOpType.add)
                                else:
                                    nc.vector.tensor_copy(tot[:, :], sums[:, 0:1])
                                rec = p2sm.tile([P, 1], FP, name="rec", tag="rec")
                                nc.vector.reciprocal(rec[:, :], tot[:, :])
                                nc.vector.tensor_scalar_mul(
                                    p_sb[:, :kend], p_sb[:, :kend], rec[:, 0:1])
                                psbs.append(p_sb)
                            # PV for this q-chunk: o^T [hd, 512 q]
                            o_ps = psB.tile([P, 512], FP, name="o_ps", tag="B")
                            for kt in range(qc * 4 + 4):
                                off = max(0, kt * P - qc * 512)
                                pt_ps = psC.tile([P, 512], BF, name="pt_ps", tag="C")
                                for j in range(4):
                                    qi = qc * 4 + j
                                    if kt <= qi:
                                        nc.tensor.transpose(
                                            pt_ps[:, j * P:(j + 1) * P],
                                            psbs[j][:, kt * P:(kt + 1) * P],
                                            ident[:, :])
                                pt_sb = p2pt.tile([P, 512], BF, name="pt_sb")
                                nc.vector.tensor_copy(pt_sb[:, off:], pt_ps[:, off:])
                                nc.tensor.matmul(
                                    o_ps[:, off:],
                                    v_sb[:, (b * N_LT + kt) * ES + h * HD:
                                         (b * N_LT + kt) * ES + (h + 1) * HD],
                                    pt_sb[:, off:],
                                    start=(kt == 0), stop=(kt == qc * 4 + 3))
                            obcp = nc.vector.tensor_copy(
                                ob_sb[:, h * L + qc * 512:h * L + (qc + 1) * 512],
                                o_ps[:, :])
                            ob_copies[(b, qc)] = obcp
                        # piece boundary: bounce + AllGather
                        for (c0, c1) in AG_PIECES[b]:
                            if c1 == qc + 1:
                                for h in range(HPC):
                                    nc.sync.dma_start(
                                        out=o_bounce[(b, c0)][h * HD:(h + 1) * HD, :],
                                        in_=ob_sb[:, h * L + c0 * 512:
                                                  h * L + c1 * 512])
                                nc.gpsimd.collective_compute(
                                    "AllGather", mybir.AluOpType.bypass,
                                    ins=[o_bounce[(b, c0)][:]],
                                    outs=[ag_o[(b, c0)][:]],
                                    replica_groups=rg)

                def phase3(b, c0, c1, dep=None):
                    w = (c1 - c0) * 512
                    for tch in range(w // 512):
                        ot_ch = p3x.tile([P, N_DT * 512], BF, name="ot_ch")
                        for dt in range(N_DT):
                            d = nc.sync.dma_start(
                                out=ot_ch[:, dt * 512:(dt + 1) * 512],
                                in_=ag_o[(b, c0)][dt * P:(dt + 1) * P,
                                                  tch * 512:(tch + 1) * 512])
                            if dep is not None and tch == 0:
                                add_dep_helper(
                                    d.ins, dep.ins,
                                    reason="stagger ph3 behind real AG")
                        t0 = b * L + (c0 + tch) * 512
                        for et in range(2):
                            f_ps = psD.tile([P, 512], FP, name="f_ps", tag="D")
                            for dt in range(N_DT):
                                nc.tensor.matmul(
                                    f_ps[:, :],
                                    woT_sb[:, dt * ES + et * P:
                                           dt * ES + (et + 1) * P],
                                    ot_ch[:, dt * 512:(dt + 1) * 512],
                                    start=(dt == 0), stop=(dt == N_DT - 1))
                            f_sb = p3o.tile([P, 512], FP, name="f_sb")
                            nc.vector.tensor_copy(f_sb[:, :], f_ps[:, :])
                            nc.sync.dma_start(
                                out=out[et * P:(et + 1) * P, t0:t0 + 512],
                                in_=f_sb[:, :])

                phase2(0)
                phase2(1)
                phase3(0, 0, 2, dep=ob_copies[(0, 3)])
                phase3(0, 2, 4, dep=ob_copies[(1, 0)])
                phase3(1, 0, 2, dep=ob_copies[(1, 2)])
                phase3(1, 2, 3, dep=ob_copies[(1, 3)])
                phase3(1, 3, 4)

            if debug:
                with tc.tile_pool(name="dbgo", bufs=2) as dbgo:
                    for b, pieces in AG_PIECES.items():
                        for (c0, c1) in pieces:
                            w = (c1 - c0) * 512
                            for i in range(HPC):
                                s = dbgo.tile([P, 2048], FP, name="dob")
                                stg = dbgo.tile([P, 2048], BF, name="dob_b")
                                nc.sync.dma_start(
                                    out=stg[:, :w],
                                    in_=o_bounce[(b, c0)][i * HD:(i + 1) * HD, :])
                                nc.vector.tensor_copy(s[:, :w], stg[:, :w])
                                nc.sync.dma_start(
                                    out=dbg_o[i * HD:(i + 1) * HD,
                                              b * L + c0 * 512:
                                              b * L + c0 * 512 + w],
                                    in_=s[:, :w])

    split_multi_waits(nc)
    return nc


def make_in_maps(x, cos, sin, Wqkv, Wo):
    bf = ml_dtypes.bfloat16
    xr = np.asarray(x).reshape(T, D)
    # q/k column permutation: head-major, evens then odds
    perm = []
    for h in range(HPC):
        perm.extend(h * HD + 2 * np.arange(64))
        perm.extend(h * HD + 2 * np.arange(64) + 1)
    perm = np.asarray(perm)
    epick = np.concatenate([h * HD + 2 * np.arange(64) for h in range(HPC)])
    in_maps = []
    for c in range(N_CORES):
        cols = slice(c * ES, (c + 1) * ES)
        wq = Wqkv[c * ES:(c + 1) * ES, :][perm]
        wk = Wqkv[D + c * ES: D + (c + 1) * ES, :][perm]
        wv = Wqkv[2 * D + c * ES: 2 * D + (c + 1) * ES, :]
        w_c = np.concatenate([wq, wk, wv], axis=0)
        cos_c = np.asarray(cos)[:, cols]
        sin_c = np.asarray(sin)[:, cols]
        in_maps.append({
            "x_c": np.ascontiguousarray(xr[c * TS:(c + 1) * TS]).astype(bf),
            "wqkvT": np.ascontiguousarray(w_c.T.astype(bf)),
            "ce_p": np.ascontiguousarray(cos_c[:, epick]).astype(np.float32),
            "co_p": np.ascontiguousarray(cos_c[:, epick + 1]).astype(np.float32),
            "se_p": np.ascontiguousarray(sin_c[:, epick]).astype(np.float32),
            "so_p": np.ascontiguousarray(sin_c[:, epick + 1]).astype(np.float32),
            "woT": np.ascontiguousarray(Wo[cols, :].T.astype(bf)),
        })
    return in_maps


_cache = {}


def kernel(x, cos, sin, Wqkv, Wo):
    from concourse.bass_utils import run_bass_kernel_spmd
    x = np.asarray(x, dtype=np.float32)
    cos = np.asarray(cos, dtype=np.float32)
    sin = np.asarray(sin, dtype=np.float32)
    Wqkv = np.asarray(Wqkv, dtype=np.float32)
    Wo = np.asarray(Wo, dtype=np.float32)
    if "nc" not in _cache:
        _cache["nc"] = build()
    nc = _cache["nc"]
    in_maps = make_in_maps(x, cos, sin, Wqkv, Wo)
    res = run_bass_kernel_spmd(nc, in_maps, core_ids=list(range(N_CORES)))
    pieces = [res.results[c]["out"].T for c in range(N_CORES)]
    return np.concatenate(pieces, axis=1).reshape(B, L, D)



# revision 34
# speedup vs baseline: 1.3235x; 1.1669x over previous
"""Distributed causal attention block (QKV + RoPE + SDPA + Wo) on 8 TRN2 cores.

Sharding: tensor-parallel over heads (2 heads/core). Each core:
  phase 1: streams full x (bf16), PE-transposes tiles inline, QKV projection
           for its 2 heads + RoPE + transposes -> QT/KT/V resident in SBUF
  phase 2: causal attention per (batch, head), q-chunk-grouped PV with the
           softmax normalization folded into a P-prescale (on GpSimd)
  phase 3: AllGather attention outputs (d-sharded, in progressive t-pieces,
           tapering at the end) -> Wo e-slice -> output
Host concatenates the 8 e-slices.

The q/k columns of Wqkv (and the cos/sin tables) are permuted head-major
even/odd on the host so RoPE runs on contiguous blocks; attention scores are
invariant to a shared permutation of the head dim of Q and K.
"""
import numpy as np
import ml_dtypes
import bass_rust
import concourse.bass as bass
import concourse.mybir as mybir
from concourse.tile import TileContext, add_dep_helper
from concourse.masks import make_identity, make_causal_mask

B, L, D, H = 2, 2048, 2048, 16
HD = 128
N_CORES = 8
HPC = H // N_CORES          # heads per core = 2
ES = HPC * HD               # 256 = e-slice width per core
T = B * L                   # 4096 tokens total
TS = T // N_CORES           # 512 t per rank-block in phase 1
P = 128
SCALE = 1.0 / float(np.sqrt(HD))
NEG = -30000.0              # causal mask fill; exp(SCALE*(s+NEG)) underflows to 0
FP = mybir.dt.float32
BF = mybir.dt.bfloat16

N_TT = T // P               # 32 global t-tiles
N_LT = L // P               # 16 t-tiles per batch
N_DT = D // P               # 16 d-tiles

# attention-out AllGather pieces per batch, in units of 512-t q-chunks (4/batch)
AG_PIECES = {0: [(0, 2), (2, 4)], 1: [(0, 3), (3, 4)]}


def split_multi_waits(nc):
    """This walrus build allows 1 sync wait per instruction (2 for
    EventSemaphore). Tile attaches more on some instructions (tail drain,
    collective-adjacent DMAs); hoist the extras onto same-engine NoOps."""
    for f in nc.m.functions:
        for bb in f.blocks:
            new_insts = []
            changed = False
            for ins in bb.instructions:
                si = ins.sync_info
                cap = 2 if type(ins).__name__ == "InstEventSemaphore" else 1
                if si is not None and len(si.on_wait) > cap:
                    waits = list(si.on_wait)
                    for k, w in enumerate(waits[cap:]):
                        new_insts.append(mybir.InstNoOp(
                            name=f"{ins.name}-wsplit{k}", ins=[], outs=[],
                            engine=ins.engine,
                            sync_info=bass_rust.SyncInfo(on_wait=[w], on_update=[]),
                        ))
                    ins.sync_info = bass_rust.SyncInfo(
                        on_wait=waits[:cap], on_update=list(si.on_update))
                    changed = True
                new_insts.append(ins)
            if changed:
                bb.instructions.clear()
                for i2 in new_insts:
                    bb.add_instruction(i2)


def build(debug=False):
    nc = bass.Bass()
    x_c = nc.declare_dram_parameter("x_c", [T, D], BF, isOutput=False)
    wqkvT = nc.declare_dram_parameter("wqkvT", [D, 3 * ES], BF, isOutput=False)
    ce_p = nc.declare_dram_parameter("ce_p", [L, P], FP, isOutput=False)
    co_p = nc.declare_dram_parameter("co_p", [L, P], FP, isOutput=False)
    se_p = nc.declare_dram_parameter("se_p", [L, P], FP, isOutput=False)
    so_p = nc.declare_dram_parameter("so_p", [L, P], FP, isOutput=False)
    woT = nc.declare_dram_parameter("woT", [D, ES], BF, isOutput=False)
    out = nc.declare_dram_parameter("out", [ES, T], FP, isOutput=True)
    if debug:
        dbg_qt = nc.declare_dram_parameter("dbg_qt", [P, HPC * T], FP, isOutput=True)
        dbg_kt = nc.declare_dram_parameter("dbg_kt", [P, HPC * T], FP, isOutput=True)
        dbg_v = nc.declare_dram_parameter("dbg_v", [T, ES], FP, isOutput=True)
        dbg_o = nc.declare_dram_parameter("dbg_o", [ES, T], FP, isOutput=True)

    # out AllGather bounce/result per (batch, piece)
    o_bounce, ag_o = {}, {}
    for b, pieces in AG_PIECES.items():
        for (c0, c1) in pieces:
            w = (c1 - c0) * 512
            o_bounce[(b, c0)] = nc.dram_tensor(f"o_bounce{b}_{c0}", [ES, w], BF)
            ag_o[(b, c0)] = nc.dram_tensor(f"ag_o{b}_{c0}", [N_CORES * ES, w], BF,
                                           addr_space="Shared")
    rg = [list(range(N_CORES))]

    def r3(ap):  # [128, 256] -> [128, 2 heads, 2 (even/odd), 64]
        return ap.rearrange("p (h s x) -> p h s x", h=2, s=2)

    def r2(ap):  # [128, 128] -> [128, 2 heads, 64]
        return ap.rearrange("p (h x) -> p h x", h=2)

    with TileContext(nc, pool_alloc_mode="queue") as tc:
        with (
            tc.tile_pool(name="const", bufs=1) as const_pool,
            tc.tile_pool(name="resident", bufs=1) as res_pool,
            tc.tile_pool(name="wo", bufs=1) as wo_pool,
            tc.tile_pool(name="psA", bufs=2, space="PSUM") as psA,
            tc.tile_pool(name="psB", bufs=2, space="PSUM") as psB,
            tc.tile_pool(name="psC", bufs=2, space="PSUM") as psC,
            tc.tile_pool(name="psD", bufs=2, space="PSUM") as psD,
        ):
            ident = const_pool.tile([P, P], BF, name="ident")
            make_identity(nc, ident[:, :])
            cmask = const_pool.tile([P, P], FP, name="cmask")
            make_causal_mask(nc, cmask[:, :], mask_val=NEG)

            # resident through phases 1-2
            qt_sb = res_pool.tile([P, HPC * T], BF, name="qt_sb")   # [hd', h*T + t]
            kt_sb = res_pool.tile([P, HPC * T], BF, name="kt_sb")
            v_sb = res_pool.tile([P, N_TT * ES], BF, name="v_sb")   # [t%128, tt*ES+e]

            # ---------------- phase 1: x^T tiles + QKV + RoPE ----------------
            with (
                tc.tile_pool(name="wq", bufs=1) as wq_pool,
                tc.tile_pool(name="p1n", bufs=8) as p1n,
                tc.tile_pool(name="p1x", bufs=2) as p1x,
                tc.tile_pool(name="p1t", bufs=3) as p1t,
            ):
                wt_sb = wq_pool.tile([P, N_DT * 3 * ES], BF, name="wt_sb")
                trig_sb = {}
                for nm in ("ce", "co", "se", "so"):
                    trig_sb[nm] = wq_pool.tile([P, N_LT * P], FP, name=f"{nm}_sb")
                woT_sb = wo_pool.tile([P, N_DT * ES], BF, name="woT_sb")

                xins = {}

                def load_xins(rb):
                    tiles = []
                    for tl in range(TS // P):
                        xin = p1n.tile([P, D], BF, name="xin", tag="xin")
                        t0 = rb * TS + tl * P
                        nc.sync.dma_start(out=xin[:, :], in_=x_c[t0:t0 + P, :])
                        tiles.append(xin)
                    xins[rb] = tiles

                # priority: first two rank blocks of x, then weights, then trig
                load_xins(0)
                load_xins(1)
                for dt in range(N_DT):
                    nc.sync.dma_start(out=wt_sb[:, dt * 3 * ES:(dt + 1) * 3 * ES],
                                      in_=wqkvT[dt * P:(dt + 1) * P, :])
                for nm, prm in (("ce", ce_p), ("co", co_p),
                                ("se", se_p), ("so", so_p)):
                    for lt in range(N_LT):
                        nc.sync.dma_start(out=trig_sb[nm][:, lt * P:(lt + 1) * P],
                                          in_=prm[lt * P:(lt + 1) * P, :])

                for rb in range(N_CORES):
                    if rb + 2 < N_CORES:
                        pass  # xins loaded lazily below
                    # build x^T tiles for this rank block on the PE
                    xt_rb = p1x.tile([P, N_DT * TS], BF, name="xt_rb")
                    for dt in range(N_DT):
                        txp = psC.tile([P, TS], BF, name="txp", tag="C")
                        for tl in range(TS // P):
                            nc.tensor.transpose(
                                txp[:, tl * P:(tl + 1) * P],
                                xins[rb][tl][:, dt * P:(dt + 1) * P], ident[:, :])
                        nc.any.tensor_copy(xt_rb[:, dt * TS:(dt + 1) * TS],
                                           txp[:, :])
                    if rb + 2 < N_CORES:
                        load_xins(rb + 2)
                    if rb == N_CORES - 1:
                        for dt in range(N_DT):
                            nc.sync.dma_start(
                                out=woT_sb[:, dt * ES:(dt + 1) * ES],
                                in_=woT[dt * P:(dt + 1) * P, :])
                    for tl in range(TS // P):
                        tt = rb * (TS // P) + tl
                        lt = tt % N_LT
                        ps_qk = psA.tile([P, 2 * ES], FP, name="ps_qk", tag="A")
                        ps_v = psB.tile([P, ES], FP, name="ps_v", tag="B")
                        for dt in range(N_DT):
                            lhsT = xt_rb[:, dt * TS + tl * P: dt * TS + (tl + 1) * P]
                            nc.tensor.matmul(
                                ps_qk[:, :], lhsT,
                                wt_sb[:, dt * 3 * ES: dt * 3 * ES + 2 * ES],
                                start=(dt == 0), stop=(dt == N_DT - 1))
                            nc.tensor.matmul(
                                ps_v[:, :], lhsT,
                                wt_sb[:, dt * 3 * ES + 2 * ES:(dt + 1) * 3 * ES],
                                start=(dt == 0), stop=(dt == N_DT - 1))
                        nc.vector.tensor_copy(v_sb[:, tt * ES:(tt + 1) * ES],
                                              ps_v[:, :])
                        ce = r2(trig_sb["ce"][:, lt * P:(lt + 1) * P])
                        co = r2(trig_sb["co"][:, lt * P:(lt + 1) * P])
                        se = r2(trig_sb["se"][:, lt * P:(lt + 1) * P])
                        so = r2(trig_sb["so"][:, lt * P:(lt + 1) * P])
                        for part in range(2):  # 0=q, 1=k
                            src = r3(ps_qk[:, part * ES:(part + 1) * ES])
                            e_, o_ = src[:, :, 0, :], src[:, :, 1, :]
                            rot = p1t.tile([P, ES], BF, name="rot", tag="rot")
                            rdst = r3(rot[:, :])
                            re_, ro_ = rdst[:, :, 0, :], rdst[:, :, 1, :]
                            t1 = p1t.tile([P, P], FP, name="t1", tag="t1")
                            t2 = p1t.tile([P, P], FP, name="t2", tag="t2")
                            t13, t23 = r2(t1[:, :]), r2(t2[:, :])
                            nc.vector.tensor_tensor(t13, e_, ce,
                                                    op=mybir.AluOpType.mult)
                            nc.vector.tensor_tensor(t23, o_, se,
                                                    op=mybir.AluOpType.mult)
                            nc.vector.tensor_tensor(re_, t13, t23,
                                                    op=mybir.AluOpType.subtract)
                            nc.vector.tensor_tensor(t13, o_, co,
                                                    op=mybir.AluOpType.mult)
                            nc.vector.tensor_tensor(t23, e_, so,
                                                    op=mybir.AluOpType.mult)
                            nc.vector.tensor_tensor(ro_, t13, t23,
                                                    op=mybir.AluOpType.add)
                            dst = qt_sb if part == 0 else kt_sb
                            for h in range(HPC):
                                tps = psD.tile([P, P], BF, name="tps", tag="D")
                                nc.tensor.transpose(
                                    tps[:, :], rot[:, h * HD:(h + 1) * HD],
                                    ident[:, :])
                                nc.vector.tensor_copy(
                                    dst[:, h * T + tt * P: h * T + (tt + 1) * P],
                                    tps[:, :])

            if debug:
                with tc.tile_pool(name="dbgp", bufs=2) as dbgp:
                    for nm, src in (("dbg_qt", qt_sb), ("dbg_kt", kt_sb)):
                        dd = {"dbg_qt": dbg_qt, "dbg_kt": dbg_kt}[nm]
                        for i in range(HPC * T // 512):
                            s = dbgp.tile([P, 512], FP, name="dstage")
                            nc.vector.tensor_copy(s[:, :],
                                                  src[:, i * 512:(i + 1) * 512])
                            nc.sync.dma_start(out=dd[:, i * 512:(i + 1) * 512],
                                              in_=s[:, :])
                    for tt in range(N_TT):
                        s = dbgp.tile([P, ES], FP, name="dstage2")
                        nc.vector.tensor_copy(s[:, :], v_sb[:, tt * ES:(tt + 1) * ES])
                        nc.sync.dma_start(out=dbg_v[tt * P:(tt + 1) * P, :],
                                          in_=s[:, :])

            # ---------------- phases 2+3 (shared pools, interleaved) ----------
            with (
                tc.tile_pool(name="p2p", bufs=6) as p2p,
                tc.tile_pool(name="p2pt", bufs=3) as p2pt,
                tc.tile_pool(name="p2sm", bufs=4) as p2sm,
                tc.tile_pool(name="p2ob", bufs=2) as p2ob,
                tc.tile_pool(name="p3x", bufs=2) as p3x,
                tc.tile_pool(name="p3o", bufs=2) as p3o,
            ):
                ob_copies = {}

                def phase2(b):
                    ob_sb = p2ob.tile([P, HPC * L], BF, name="ob_sb", tag="ob")
                    for qc in range(4):
                        for h in range(HPC):
                            qoff = h * T + b * L
                            psbs = []
                            for qi in range(qc * 4, qc * 4 + 4):
                                kend = (qi + 1) * P
                                nch = (kend + 511) // 512
                                p_sb = p2p.tile([P, L], BF, name="p_sb", tag="p")
                                sums = p2sm.tile([P, 4], FP, name="sums", tag="sums")
                                for ci in range(nch):
                                    klo = ci * 512
                                    ksz = min(512, kend - klo)
                                    s_ps = psA.tile([P, 512], FP, name="s_ps", tag="A")
                                    nc.tensor.matmul(
                                        s_ps[:, :ksz],
                                        qt_sb[:, qoff + qi * P: qoff + (qi + 1) * P],
                                        kt_sb[:, qoff + klo: qoff + klo + ksz],
                                        start=True, stop=True)
                                    if klo + ksz == kend:  # diagonal 128-block
                                        dslice = s_ps[:, ksz - P:ksz]
                                        nc.vector.tensor_tensor(
                                            dslice, dslice, cmask[:, :],
                                            op=mybir.AluOpType.add)
                                    nc.scalar.activation(
                                        p_sb[:, klo:klo + ksz], s_ps[:, :ksz],
                                        mybir.ActivationFunctionType.Exp,
                                        scale=SCALE,
                                        accum_out=sums[:, ci:ci + 1])
                                tot = p2sm.tile([P, 1], FP, name="tot", tag="tot")
                                if nch > 1:
                                    nc.vector.tensor_reduce(
                                        tot[:, :], sums[:, :nch],
                                        axis=mybir.AxisListType.X,
                                        op=mybir.AluOpType.add)
                                else:
                                    nc.vector.tensor_copy(tot[:, :], sums[:, 0:1])
                                rec = p2sm.tile([P, 1], FP, name="rec", tag="rec")
                                nc.vector.reciprocal(rec[:, :], tot[:, :])
                                nc.vector.tensor_scalar_mul(
                                    p_sb[:, :kend], p_sb[:, :kend], rec[:, 0:1])
                                psbs.append(p_sb)
                            # PV for this q-chunk: o^T [hd, 512 q]
                            o_ps = psB.tile([P, 512], FP, name="o_ps", tag="B")
                            for kt in range(qc * 4 + 4):
                                off = max(0, kt * P - qc * 512)
                                pt_ps = psC.tile([P, 512], BF, name="pt_ps", tag="C")
                                for j in range(4):
                                    qi = qc * 4 + j
                                    if kt <= qi:
                                        nc.tensor.transpose(
                                            pt_ps[:, j * P:(j + 1) * P],
                                            psbs[j][:, kt * P:(kt + 1) * P],
                                            ident[:, :])
                                pt_sb = p2pt.tile([P, 512], BF, name="pt_sb")
                                nc.vector.tensor_copy(pt_sb[:, off:], pt_ps[:, off:])
                                nc.tensor.matmul(
                                    o_ps[:, off:],
                                    v_sb[:, (b * N_LT + kt) * ES + h * HD:
                                         (b * N_LT + kt) * ES + (h + 1) * HD],
                                    pt_sb[:, off:],
                                    start=(kt == 0), stop=(kt == qc * 4 + 3))
                            obcp = nc.vector.tensor_copy(
                                ob_sb[:, h * L + qc * 512:h * L + (qc + 1) * 512],
                                o_ps[:, :])
                            ob_copies[(b, qc)] = obcp
                        # piece boundary: bounce + AllGather
                        for (c0, c1) in AG_PIECES[b]:
                            if c1 == qc + 1:
                                for h in range(HPC):
                                    nc.sync.dma_start(
                                        out=o_bounce[(b, c0)][h * HD:(h + 1) * HD, :],
                                        in_=ob_sb[:, h * L + c0 * 512:
                                                  h * L + c1 * 512])
                                nc.gpsimd.collective_compute(
                                    "AllGather", mybir.AluOpType.bypass,
                                    ins=[o_bounce[(b, c0)][:]],
                                    outs=[ag_o[(b, c0)][:]],
                                    replica_groups=rg)

                def phase3(b, c0, c1, dep=None):
                    w = (c1 - c0) * 512
                    for tch in range(w // 512):
                        ot_ch = p3x.tile([P, N_DT * 512], BF, name="ot_ch")
                        for dt in range(N_DT):
                            d = nc.sync.dma_start(
                                out=ot_ch[:, dt * 512:(dt + 1) * 512],
                                in_=ag_o[(b, c0)][dt * P:(dt + 1) * P,
                                                  tch * 512:(tch + 1) * 512])
                            if dep is not None and tch == 0:
                                add_dep_helper(
                                    d.ins, dep.ins,
                                    reason="stagger ph3 behind real AG")
                        t0 = b * L + (c0 + tch) * 512
                        for et in range(2):
                            f_ps = psD.tile([P, 512], FP, name="f_ps", tag="D")
                            for dt in range(N_DT):
                                nc.tensor.matmul(
                                    f_ps[:, :],
                                    woT_sb[:, dt * ES + et * P:
                                           dt * ES + (et + 1) * P],
                                    ot_ch[:, dt * 512:(dt + 1) * 512],
                                    start=(dt == 0), stop=(dt == N_DT - 1))
                            f_sb = p3o.tile([P, 512], FP, name="f_sb")
                            nc.vector.tensor_copy(f_sb[:, :], f_ps[:, :])
                            nc.sync.dma_start(
                                out=out[et * P:(et + 1) * P, t0:t0 + 512],
                                in_=f_sb[:, :])

                phase2(0)
                phase2(1)
                phase3(0, 0, 2, dep=ob_copies[(0, 3)])
                phase3(0, 2, 4, dep=ob_copies[(1, 0)])
                phase3(1, 0, 3, dep=ob_copies[(1, 2)])
                phase3(1, 3, 4)

            if debug:
                with tc.tile_pool(name="dbgo", bufs=2) as dbgo:
                    for b, pieces in AG_PIECES.items():
                        for (c0, c1) in pieces:
                            w = (c1 - c0) * 512
                            for i in range(HPC):
                                s = dbgo.tile([P, 2048], FP, name="dob")
                                stg = dbgo.tile([P, 2048], BF, name="dob_b")
                                nc.sync.dma_start(
                                    out=stg[:, :w],
                                    in_=o_bounce[(b, c0)][i * HD:(i + 1) * HD, :])
                                nc.vector.tensor_copy(s[:, :w], stg[:, :w])
                                nc.sync.dma_start(
                                    out=dbg_o[i * HD:(i + 1) * HD,
                                              b * L + c0 * 512:
                                              b * L + c0 * 512 + w],
                                    in_=s[:, :w])

    split_multi_waits(nc)
    return nc


def make_in_maps(x, cos, sin, Wqkv, Wo):
    bf = ml_dtypes.bfloat16
    xf = np.ascontiguousarray(np.asarray(x).reshape(T, D)).astype(bf)
    # q/k column permutation: head-major, evens then odds
    perm = []
    for h in range(HPC):
        perm.extend(h * HD + 2 * np.arange(64))
        perm.extend(h * HD + 2 * np.arange(64) + 1)
    perm = np.asarray(perm)
    epick = np.concatenate([h * HD + 2 * np.arange(64) for h in range(HPC)])
    in_maps = []
    for c in range(N_CORES):
        cols = slice(c * ES, (c + 1) * ES)
        wq = Wqkv[c * ES:(c + 1) * ES, :][perm]
        wk = Wqkv[D + c * ES: D + (c + 1) * ES, :][perm]
        wv = Wqkv[2 * D + c * ES: 2 * D + (c + 1) * ES, :]
        w_c = np.concatenate([wq, wk, wv], axis=0)
        cos_c = np.asarray(cos)[:, cols]
        sin_c = np.asarray(sin)[:, cols]
        in_maps.append({
            "x_c": xf,
            "wqkvT": np.ascontiguousarray(w_c.T.astype(bf)),
            "ce_p": np.ascontiguousarray(cos_c[:, epick]).astype(np.float32),
            "co_p": np.ascontiguousarray(cos_c[:, epick + 1]).astype(np.float32),
            "se_p": np.ascontiguousarray(sin_c[:, epick]).astype(np.float32),
            "so_p": np.ascontiguousarray(sin_c[:, epick + 1]).astype(np.float32),
            "woT": np.ascontiguousarray(Wo[cols, :].T.astype(bf)),
        })
    return in_maps


_cache = {}


def kernel(x, cos, sin, Wqkv, Wo):
    from concourse.bass_utils import run_bass_kernel_spmd
    x = np.asarray(x, dtype=np.float32)
    cos = np.asarray(cos, dtype=np.float32)
    sin = np.asarray(sin, dtype=np.float32)
    Wqkv = np.asarray(Wqkv, dtype=np.float32)
    Wo = np.asarray(Wo, dtype=np.float32)
    if "nc" not in _cache:
        _cache["nc"] = build()
    nc = _cache["nc"]
    in_maps = make_in_maps(x, cos, sin, Wqkv, Wo)
    res = run_bass_kernel_spmd(nc, in_maps, core_ids=list(range(N_CORES)))
    pieces = [res.results[c]["out"].T for c in range(N_CORES)]
    return np.concatenate(pieces, axis=1).reshape(B, L, D)

